# revision 1
# baseline (speedup 1.0000x reference)
"""Trainium2 Bass kernel for nn_ContrastiveLoss (stacked cross-attention t2i).

Strategy (8 NeuronCores, caption-sharded):
  - Each core owns 16 of the 128 captions and all 128 images.
  - Per batch of 3 images x 16 captions, compute A = im @ s^T via PE (f32r),
    the two softmaxes (word softmax normalized; region softmax's normalizer
    cancels inside cosine similarity, so only exp(9*a1) is needed), the
    cosine numerator/denominator via PE column sums, and stage per-word
    results into a [128, 800] tile.
  - One finalization pass turns staged tiles into the [128, 16] score block.
  - AllGather score blocks -> every core holds scores [128, 128]; the hinge
    margin loss (max violation) is computed on-device; host reads the scalar.

Math note: with E2 = exp(lam * a1) (unnormalized region attention),
  cos = (sum_r E2*A) / (cap_n * sqrt(E2^T G E2)) exactly, because the region
softmax normalizer cancels between numerator and |weighted context|.
"""

import numpy as np

import concourse.bass as bass
import concourse.tile as tile
from concourse import mybir
from concourse.bass_utils import run_bass_kernel_spmd
from concourse.vector_clock import ScopedClock

# ---------------------------------------------------------------------------
# Workaround for this toolchain: walrus rejects instructions carrying more
# than one semaphore wait.  Split extra waits onto standalone EventSemaphore
# instructions (the same thing wait_ge emits) just before the offender.
# ---------------------------------------------------------------------------
_PATCHED = False


def _install_patches():
    global _PATCHED
    if _PATCHED:
        return
    _PATCHED = True

    def _drain_and_barrier(self, tick_clock, wait_clock):
        nc = self.nc
        drain_inst = nc.sync.drain()
        wait_clock.add_sem_waits(
            drain_inst.ins, ScopedClock({None: tick_clock.global_clock})
        )
        waits = list(drain_inst.ins.sync_info.on_wait)
        if len(waits) > 1:
            drain_inst.ins.sync_info.on_wait = waits[:1]
            for w in waits[1:]:
                extra = nc.sync.drain()
                extra.ins.sync_info = mybir.SyncInfo(on_wait=[w], on_update=[])
        nc.all_engine_barrier()
        popped = nc._tile_sem_poison_stack.pop()
        assert popped is self._sem_poison
        nc.clear_and_free_semaphores(list(self.sems.allocated().values()))
        nc.all_engine_barrier()

    tile.TileContext._drain_and_barrier = _drain_and_barrier

    import concourse.bass_utils as bass_utils
    import concourse.bass2jax as bass2jax
    import orjson

    _orig_compile = bass_utils.compile_bir_kernel

    def _split_waits_in_bir(bir_json: bytes) -> bytes:
        m = orjson.loads(bir_json)
        for fn in m.get("functions", []):
            for blk in fn.get("blocks", []):
                insts = blk.get("instructions", [])
                new_insts = []
                for ins in insts:
                    si = ins.get("sync_info")
                    waits = (si or {}).get("on_wait") or []
                    if len(waits) > 1:
                        for k, w in enumerate(waits[:-1]):
                            new_insts.append(
                                {
                                    "name": f"{ins['name']}_wsplit{k}",
                                    "opcode": "EventSemaphore",
                                    "engine": ins["engine"],
                                    "ins": [],
                                    "outs": [],
                                    "debug": ins.get("debug"),
                                    "sync_info": {"on_update": [], "on_wait": [w]},
                                }
                            )
                        si["on_wait"] = waits[-1:]
                    new_insts.append(ins)
                blk["instructions"] = new_insts
        return orjson.dumps(m)

    def _patched_compile(bir_json, tmpdir, neff_name="file.neff"):
        return _orig_compile(_split_waits_in_bir(bir_json), tmpdir, neff_name)

    bass_utils.compile_bir_kernel = _patched_compile
    bass2jax.compile_bir_kernel = _patched_compile


# ---------------------------------------------------------------------------
# Problem constants (hardcoded per the task contract).
# ---------------------------------------------------------------------------
B = 128           # images == captions
LI = 36           # image regions
LW = 50           # padded caption words
D = 1024          # feature dim
NC = 8            # cores
CAP = B // NC     # captions per core (16)
WF = CAP * LW     # free width of the batched tiles (800)
IMG_GRP = 3       # images per batch
NB = (B + IMG_GRP - 1) // IMG_GRP  # 43 batches (42x3 + 1x2)
LAM = 9.0
MARGIN = 0.2
EPS = 1e-8
MASKNEG = -30000.0

F32 = mybir.dt.float32
F32R = mybir.dt.float32r

# When True, subtract a per-(row, caption)-segment max before the word
# softmax (exactly like the reference).  When False, use a per-row global max
# as the exp offset (one fewer pass; softmax value is identical unless an
# entire segment underflows).
SEGMAX = True

_CACHE = {}


def _build_program():
    nc = bass.Bass("TRN2", target_bir_lowering=False, debug=False, num_devices=NC)

    # Inputs (per-core contents differ only for sT8 / mask / wfac).
    imT8 = nc.dram_tensor("imT8", [8, 128, B * LI], F32R, kind="ExternalInput")
    sT8 = nc.dram_tensor("sT8", [8, 128, WF], F32R, kind="ExternalInput")
    g43 = nc.dram_tensor("g43", [NB, IMG_GRP * LI, IMG_GRP * LI], F32R, kind="ExternalInput")
    maskneg_d = nc.dram_tensor("maskneg", [1, WF], F32R, kind="ExternalInput")
    wfac_d = nc.dram_tensor("wfac", [128, WF], F32, kind="ExternalInput")
    eye_d = nc.dram_tensor("eye", [128, 128], F32, kind="ExternalInput")
    noteye_d = nc.dram_tensor("noteye", [128, 128], F32, kind="ExternalInput")
    onesblk_d = nc.dram_tensor("onesblk", [IMG_GRP * LI, IMG_GRP], F32R, kind="ExternalInput")
    ones1_d = nc.dram_tensor("ones1", [1, IMG_GRP * LI], F32R, kind="ExternalInput")
    ones128_d = nc.dram_tensor("ones128", [128, 1], F32R, kind="ExternalInput")

    loss_out = nc.dram_tensor("loss_out", [1, 2], F32, kind="ExternalOutput")
    scores_out = nc.dram_tensor("scores_out", [128, 128], F32, kind="ExternalOutput")

    with tile.TileContext(nc) as tc:
        with (
            tc.tile_pool(name="const", bufs=1) as cpool,
            tc.tile_pool(name="imp", bufs=3) as impool,
            tc.tile_pool(name="gp", bufs=2) as gpool,
            tc.tile_pool(name="work", bufs=2) as work,
            tc.tile_pool(name="small", bufs=2) as small,
            tc.tile_pool(name="stage", bufs=1) as stage,
            tc.tile_pool(name="pa", bufs=2, space="PSUM") as pa,
            tc.tile_pool(name="pc", bufs=2, space="PSUM") as pc,
            tc.tile_pool(name="dram", bufs=1, space="DRAM") as dram,
        ):
            # ---- persistent tiles -------------------------------------------------
            sT = cpool.tile([128, 8, WF], F32R, tag="sT")
            nc.sync.dma_start(sT[:], sT8[:].transpose([1, 0, 2]))
            masknegt = cpool.tile([1, WF], F32R, tag="mn")
            nc.sync.dma_start(masknegt[:], maskneg_d[:])
            wfact = cpool.tile([128, WF], F32, tag="wf")
            nc.sync.dma_start(wfact[:], wfac_d[:])
            eyet = cpool.tile([128, 128], F32, tag="eye")
            nc.sync.dma_start(eyet[:], eye_d[:])
            noteyet = cpool.tile([128, 128], F32, tag="neye")
            nc.sync.dma_start(noteyet[:], noteye_d[:])
            onesblkt = cpool.tile([IMG_GRP * LI, IMG_GRP], F32R, tag="ob")
            nc.sync.dma_start(onesblkt[:], onesblk_d[:])
            ones1t = cpool.tile([1, IMG_GRP * LI], F32R, tag="o1")
            nc.sync.dma_start(ones1t[:], ones1_d[:])
            ones128t = cpool.tile([128, 1], F32R, tag="o128")
            nc.sync.dma_start(ones128t[:], ones128_d[:])

            nst = stage.tile([128, WF], F32, tag="nst")
            wst = stage.tile([128, WF], F32, tag="wst")

            NCH = [(0, 512), (512, WF)]

            # ---- main loop over image groups -------------------------------------
            for b in range(NB):
                ng = min(IMG_GRP, B - b * IMG_GRP)   # images in this group
                P = ng * LI                          # partitions used

                imb = impool.tile([128, 8, P], F32R, tag="imb")
                nc.sync.dma_start(
                    imb[:], imT8[:, :, b * IMG_GRP * LI : b * IMG_GRP * LI + P].transpose([1, 0, 2])
                )
                gt = gpool.tile([P, P], F32R, tag="gt")
                nc.sync.dma_start(gt[:], g43[b, 0:P, 0:P])

                # A[P, WF] = sum_c imb_c^T @ sT_c  (+ word mask row)
                a_ps = pa.tile([P, WF], F32, tag="AT")
                for n0, n1 in NCH:
                    for c in range(8):
                        nc.tensor.matmul(
                            a_ps[:, n0:n1], imb[:, c, :], sT[:, c, n0:n1],
                            start=(c == 0), stop=False,
                        )
                    nc.tensor.matmul(
                        a_ps[:, n0:n1], ones1t[0:1, 0:P], masknegt[0:1, n0:n1],
                        start=False, stop=True,
                    )

                am = work.tile([P, WF], F32, tag="am")
                nc.scalar.copy(am[:], a_ps[:])
                e = work.tile([P, WF], F32, tag="e")
                if SEGMAX:
                    mx = small.tile([P, CAP], F32, tag="mx")
                    nc.vector.tensor_reduce(
                        mx[:], a_ps[:].rearrange("p (c w) -> p c w", c=CAP, w=LW),
                        axis=mybir.AxisListType.X, op=mybir.AluOpType.max,
                    )
                    sub = work.tile([P, WF], F32, tag="sub")
                    nc.gpsimd.tensor_tensor(
                        sub[:].rearrange("p (c w) -> p c w", c=CAP, w=LW),
                        am[:].rearrange("p (c w) -> p c w", c=CAP, w=LW),
                        mx[:].unsqueeze(2).broadcast_to([P, CAP, LW]),
                        op=mybir.AluOpType.subtract,
                    )
                    nc.scalar.activation(e[:], sub[:], mybir.ActivationFunctionType.Exp)
                else:
                    negmax = small.tile([P, 1], F32, tag="negmax")
                    nc.vector.tensor_reduce(
                        negmax[:], a_ps[:], axis=mybir.AxisListType.X,
                        op=mybir.AluOpType.max, negate=True,
                    )
                    nc.scalar.activation(
                        e[:], a_ps[:], mybir.ActivationFunctionType.Exp,
                        bias=negmax[:], scale=1.0,
                    )

                z = small.tile([P, CAP], F32, tag="z")
                nc.vector.tensor_reduce(
                    z[:], e[:].rearrange("p (c w) -> p c w", c=CAP, w=LW),
                    axis=mybir.AxisListType.X, op=mybir.AluOpType.add,
                )
                rz = small.tile([P, CAP], F32, tag="rz")
                nc.vector.reciprocal(rz[:], z[:])

                m = work.tile([P, WF], F32, tag="m")
                nc.vector.tensor_tensor(
                    m[:].rearrange("p (c w) -> p c w", c=CAP, w=LW),
                    e[:].rearrange("p (c w) -> p c w", c=CAP, w=LW),
                    rz[:].unsqueeze(2).broadcast_to([P, CAP, LW]),
                    op=mybir.AluOpType.mult,
                )
                e2 = work.tile([P, WF], F32R, tag="e2")
                nc.scalar.activation(
                    e2[:], m[:], mybir.ActivationFunctionType.Exp, bias=0.0, scale=LAM
                )

                f = work.tile([P, WF], F32R, tag="f")
                nc.gpsimd.tensor_tensor(f[:], am[:], e2[:], op=mybir.AluOpType.mult)

                t_ps = pa.tile([P, WF], F32, tag="AT")
                for n0, n1 in NCH:
                    nc.tensor.matmul(t_ps[:, n0:n1], gt[:], e2[:, n0:n1], start=True, stop=True)

                u = work.tile([P, WF], F32R, tag="u")
                nc.vector.tensor_tensor(u[:], t_ps[:], e2[:], op=mybir.AluOpType.mult)

                n_ps = pc.tile([ng, WF], F32, tag="cs")
                for n0, n1 in NCH:
                    nc.tensor.matmul(n_ps[:, n0:n1], onesblkt[0:P, 0:ng], f[:, n0:n1], start=True, stop=True)
                w_ps = pc.tile([ng, WF], F32, tag="cs")
                for n0, n1 in NCH:
                    nc.tensor.matmul(w_ps[:, n0:n1], onesblkt[0:P, 0:ng], u[:, n0:n1], start=True, stop=True)

                r0 = b * IMG_GRP
                nb_sb = small.tile([ng, WF], F32, tag="nb_sb")
                wb_sb = small.tile([ng, WF], F32, tag="wb_sb")
                nc.scalar.copy(nb_sb[:], n_ps[:])
                nc.scalar.copy(wb_sb[:], w_ps[:])
                nc.sync.dma_start(nst[r0 : r0 + ng, :], nb_sb[:])
                nc.sync.dma_start(wst[r0 : r0 + ng, :], wb_sb[:])

            # ---- finalize: scores block [128 images, 16 captions] ----------------
            srt = work.tile([128, WF], F32, tag="am")
            nc.scalar.sqrt(srt[:], wst[:])
            q = work.tile([128, WF], F32, tag="e")
            nc.vector.tensor_tensor(q[:], nst[:], wfact[:], op=mybir.AluOpType.mult)
            rsq = work.tile([128, WF], F32, tag="sub" if SEGMAX else "f")
            nc.vector.reciprocal(rsq[:], srt[:])
            cosq = work.tile([128, WF], F32, tag="m")
            nc.vector.tensor_tensor(cosq[:], q[:], rsq[:], op=mybir.AluOpType.mult)
            sim = small.tile([128, CAP], F32, tag="sim")
            nc.vector.tensor_reduce(
                sim[:], cosq[:].rearrange("p (c w) -> p c w", c=CAP, w=LW),
                axis=mybir.AxisListType.X, op=mybir.AluOpType.add,
            )

            # ---- all-gather the score columns ------------------------------------
            ag_in = dram.tile([128, CAP], F32)
            ag_out = dram.tile([NC, 128, CAP], F32, addr_space="Shared")
            nc.sync.dma_start(ag_in[:], sim[:])
            nc.gpsimd.collective_compute(
                "AllGather",
                mybir.AluOpType.bypass,
                replica_groups=[list(range(NC))],
                ins=[ag_in.opt()],
                outs=[ag_out.opt()],
            )
            s_t = cpool.tile([128, NC, CAP], F32, tag="scores")
            nc.sync.dma_start(s_t[:], ag_out[:].transpose([1, 0, 2]))
            s2d = s_t[:].rearrange("p c w -> p (c w)")
            nc.sync.dma_start(scores_out[:], s2d)

            # ---- margin loss (every core computes it; core 0's is read) ----------
            junk = work.tile([128, 128], F32, tag="am")
            diag = small.tile([128, 1], F32, tag="diag")
            nc.vector.tensor_tensor(junk[:, 0:128], s2d, eyet[:], op=mybir.AluOpType.mult)
            nc.vector.tensor_reduce(
                diag[:], junk[:, 0:128], axis=mybir.AxisListType.X, op=mybir.AluOpType.add
            )
            bias = small.tile([128, 1], F32, tag="bias")
            nc.vector.tensor_scalar(
                bias[:], diag[:], scalar1=-1.0, scalar2=MARGIN,
                op0=mybir.AluOpType.mult, op1=mybir.AluOpType.add,
            )
            # cost_s = relu(S + margin - d_i), diagonal zeroed
            cs = work.tile([128, 128], F32, tag="e")
            nc.scalar.activation(
                cs[:], s2d, mybir.ActivationFunctionType.Relu, bias=bias[:], scale=1.0
            )
            cs2 = work.tile([128, 128], F32, tag="m")
            nc.vector.tensor_tensor(cs2[:], cs[:], noteyet[:], op=mybir.AluOpType.mult)
            rmaxs = small.tile([128, 2], F32R, tag="rmaxs")
            nc.vector.tensor_reduce(
                rmaxs[:, 0:1], cs2[:], axis=mybir.AxisListType.X, op=mybir.AluOpType.max
            )
            # transposed scores for cost_im
            st_ps = pc.tile([128, 128], F32, tag="cs")
            nc.tensor.transpose(st_ps[:], s_t[:].rearrange("p c w -> p (c w)"), eyet[:])
            ct = work.tile([128, 128], F32, tag="u")
            nc.scalar.activation(
                ct[:], st_ps[:], mybir.ActivationFunctionType.Relu, bias=bias[:], scale=1.0
            )
            ct2 = work.tile([128, 128], F32, tag="f")
            nc.vector.tensor_tensor(ct2[:], ct[:], noteyet[:], op=mybir.AluOpType.mult)
            nc.vector.tensor_reduce(
                rmaxs[:, 1:2], ct2[:], axis=mybir.AxisListType.X, op=mybir.AluOpType.max
            )
            tot_ps = pc.tile([1, 2], F32, tag="cs")
            nc.tensor.matmul(tot_ps[:], ones128t[:], rmaxs[:], start=True, stop=True)
            tot = small.tile([1, 2], F32, tag="tot")
            nc.scalar.copy(tot[:], tot_ps[:])
            nc.sync.dma_start(loss_out[:], tot[:])

    return nc


def _host_prep(im, s, s_l):
    im = np.ascontiguousarray(im, dtype=np.float32)
    s = np.ascontiguousarray(s, dtype=np.float32)
    s_l = np.asarray(s_l).astype(np.int64)

    # imT8[c, d, i*LI+r] = im[i, r, c*128+d]
    imT = im.reshape(B * LI, D).T            # [D, B*LI]
    imT8 = np.ascontiguousarray(imT.reshape(8, 128, B * LI))

    # gram matrices, block-diagonal per image group
    G = np.matmul(im, im.transpose(0, 2, 1))  # [B, LI, LI]
    g43 = np.zeros((NB, IMG_GRP * LI, IMG_GRP * LI), dtype=np.float32)
    for b in range(NB):
        ng = min(IMG_GRP, B - b * IMG_GRP)
        for g in range(ng):
            g43[b, g * LI : (g + 1) * LI, g * LI : (g + 1) * LI] = G[b * IMG_GRP + g]

    eye = np.eye(128, dtype=np.float32)
    noteye = 1.0 - eye
    onesblk = np.zeros((IMG_GRP * LI, IMG_GRP), dtype=np.float32)
    for g in range(IMG_GRP):
        onesblk[g * LI : (g + 1) * LI, g] = 1.0
    ones1 = np.ones((1, IMG_GRP * LI), dtype=np.float32)
    ones128 = np.ones((128, 1), dtype=np.float32)

    wmask_all = (np.arange(LW)[None, :] < s_l[:, None]).astype(np.float32)  # [B, LW]
    capn_all = np.linalg.norm(s, axis=-1)                                    # [B, LW]

    in_maps = []
    for core in range(NC):
        j0 = core * CAP
        sj = s[j0 : j0 + CAP]                       # [CAP, LW, D]
        sT = sj.reshape(WF, D).T                    # [D, WF]
        sT8 = np.ascontiguousarray(sT.reshape(8, 128, WF))
        wm = wmask_all[j0 : j0 + CAP]               # [CAP, LW]
        capn = capn_all[j0 : j0 + CAP]
        maskneg = ((1.0 - wm) * MASKNEG).reshape(1, WF).astype(np.float32)
        lens = s_l[j0 : j0 + CAP].astype(np.float32)[:, None]
        wfac = (wm / (np.maximum(capn, EPS) * lens)).reshape(WF).astype(np.float32)
        wfac = np.broadcast_to(wfac, (128, WF)).copy()
        in_maps.append(
            {
                "imT8": imT8,
                "sT8": sT8,
                "g43": g43,
                "maskneg": maskneg,
                "wfac": wfac,
                "eye": eye,
                "noteye": noteye,
                "onesblk": onesblk,
                "ones1": ones1,
                "ones128": ones128,
            }
        )
    return in_maps


def run(im, s, s_l, trace=False):
    """Returns (loss_scalar, scores[128,128], bass_results)."""
    _install_patches()
    if "nc" not in _CACHE:
        _CACHE["nc"] = _build_program()
    nc = _CACHE["nc"]
    in_maps = _host_prep(im, s, s_l)
    try:
        res = run_bass_kernel_spmd(nc, in_maps, list(range(NC)), trace=trace)
    except ModuleNotFoundError:
        # NTFF profile hook unavailable in this image; run without tracing.
        res = run_bass_kernel_spmd(nc, in_maps, list(range(NC)), trace=False)
    r0 = res.results[0]
    loss = np.float32(r0["loss_out"][0, 0] + r0["loss_out"][0, 1])
    return loss, r0["scores_out"], res


def kernel(im, s, s_l):
    loss, _, _ = run(im, s, s_l)
    return np.array(loss, dtype=np.float32)



# revision 9
# speedup vs baseline: 5.1220x; 5.1220x over previous
"""Trainium2 Bass kernel for nn_ContrastiveLoss (stacked cross-attention t2i).

Strategy (8 NeuronCores, caption-sharded; transfer-optimized):
  The axon loopback relay moves host<->device data at only ~38 MB/s, so the
  end-to-end time is dominated by upload bytes.  This version uploads one
  fp16 array per core (~2.8 MB: its 16 images + its 16 captions + two aux
  rows) instead of the ~25 MB fp32 the old kernel shipped:
    - im is uploaded SHARDED (1/8 per core) and AllGathered on-device over
      NeuronLink.
    - s stays sharded (each core only scores its own 16 captions).
    - transposes (im, s -> D-major), per-image Gram matrices, caption-word
      norms, and the wfac broadcast are all computed on device.
    - A- and T-matmuls run in fp16 (PSUM accumulates fp32); the softmax /
      cosine chain stays fp32, identical in structure to the reference.
  Each core returns its (128 x 16) score columns; the host assembles the
  full 128x128 score matrix and computes the (trivial) hinge margin loss.

Math note: with E2 = exp(lam * a1) (unnormalized region attention),
  cos = (sum_r E2*A) / (cap_n * sqrt(E2^T G E2)) exactly, because the region
softmax normalizer cancels between numerator and |weighted context|.
"""

import numpy as np

import concourse.bass as bass
import concourse.tile as tile
from concourse import mybir
from concourse.vector_clock import ScopedClock

# ---------------------------------------------------------------------------
# Workaround for this toolchain: walrus rejects instructions carrying more
# than one semaphore wait.  Split extra waits onto standalone EventSemaphore
# instructions (the same thing wait_ge emits) just before the offender.
# ---------------------------------------------------------------------------
_PATCHED = False


def _install_patches():
    global _PATCHED
    if _PATCHED:
        return
    _PATCHED = True

    def _drain_and_barrier(self, tick_clock, wait_clock):
        nc = self.nc
        drain_inst = nc.sync.drain()
        wait_clock.add_sem_waits(
            drain_inst.ins, ScopedClock({None: tick_clock.global_clock})
        )
        waits = list(drain_inst.ins.sync_info.on_wait)
        if len(waits) > 1:
            drain_inst.ins.sync_info.on_wait = waits[:1]
            for w in waits[1:]:
                extra = nc.sync.drain()
                extra.ins.sync_info = mybir.SyncInfo(on_wait=[w], on_update=[])
        nc.all_engine_barrier()
        popped = nc._tile_sem_poison_stack.pop()
        assert popped is self._sem_poison
        nc.clear_and_free_semaphores(list(self.sems.allocated().values()))
        nc.all_engine_barrier()

    tile.TileContext._drain_and_barrier = _drain_and_barrier

    import concourse.bass_utils as bass_utils
    import concourse.bass2jax as bass2jax
    import orjson

    _orig_compile = bass_utils.compile_bir_kernel

    def _split_waits_in_bir(bir_json: bytes) -> bytes:
        m = orjson.loads(bir_json)
        for fn in m.get("functions", []):
            for blk in fn.get("blocks", []):
                insts = blk.get("instructions", [])
                new_insts = []
                for ins in insts:
                    si = ins.get("sync_info")
                    waits = (si or {}).get("on_wait") or []
                    if len(waits) > 1:
                        for k, w in enumerate(waits[:-1]):
                            new_insts.append(
                                {
                                    "name": f"{ins['name']}_wsplit{k}",
                                    "opcode": "EventSemaphore",
                                    "engine": ins["engine"],
                                    "ins": [],
                                    "outs": [],
                                    "debug": ins.get("debug"),
                                    "sync_info": {"on_update": [], "on_wait": [w]},
                                }
                            )
                        si["on_wait"] = waits[-1:]
                    new_insts.append(ins)
                blk["instructions"] = new_insts
        return orjson.dumps(m)

    def _patched_compile(bir_json, tmpdir, neff_name="file.neff"):
        return _orig_compile(_split_waits_in_bir(bir_json), tmpdir, neff_name)

    bass_utils.compile_bir_kernel = _patched_compile
    bass2jax.compile_bir_kernel = _patched_compile


# ---------------------------------------------------------------------------
# Problem constants (hardcoded per the task contract).
# ---------------------------------------------------------------------------
B = 128           # images == captions
LI = 36           # image regions
LW = 50           # padded caption words
D = 1024          # feature dim
NC = 8            # cores
CAP = B // NC     # captions per core (16)
WF = CAP * LW     # free width of the batched tiles (800)
IMG_GRP = 3       # images per batch
NB = (B + IMG_GRP - 1) // IMG_GRP  # 43 batches (42x3 + 1x2)
IMROWS = CAP * LI         # 576 rows of this core's images
SROWS = CAP * LW          # 800 rows of this core's captions
COMBO_ROWS = IMROWS + SROWS + 2   # + maskneg row + wdl row
LAM = 9.0
MARGIN = 0.2
MASKNEG = -30000.0

F32 = mybir.dt.float32
F32R = mybir.dt.float32r
F16 = mybir.dt.float16

NCH = [(0, 512), (512, WF)]

_CACHE = {}


def _build_program():
    nc = bass.Bass("TRN2", target_bir_lowering=False, debug=False, num_devices=NC)

    combo = nc.dram_tensor("combo", [COMBO_ROWS, D], F16, kind="ExternalInput")
    scores_out = nc.dram_tensor("scores_out", [128, CAP], F32, kind="ExternalOutput")

    with tile.TileContext(nc) as tc:
        with (
            tc.tile_pool(name="const", bufs=1) as cpool,
            tc.tile_pool(name="nat", bufs=2) as natp,
            tc.tile_pool(name="work", bufs=2) as work,
            tc.tile_pool(name="small", bufs=2) as small,
            tc.tile_pool(name="stage", bufs=1) as stage,
            tc.tile_pool(name="pa", bufs=2, space="PSUM") as pa,
            tc.tile_pool(name="pc", bufs=2, space="PSUM") as pc,
            tc.tile_pool(name="dram", bufs=1, space="DRAM") as dram,
        ):
            # ---- on-device constants -----------------------------------------
            ones32 = cpool.tile([128, 128], F32, tag="ones32")
            nc.vector.memset(ones32[:], 1.0)
            eye32 = cpool.tile([128, 128], F32, tag="eye32")
            nc.gpsimd.affine_select(
                eye32[:], ones32[:], pattern=[[-1, 128]],
                compare_op=mybir.AluOpType.is_equal, fill=0.0,
                base=0, channel_multiplier=1,
            )
            eye16 = cpool.tile([128, 128], F16, tag="eye16")
            nc.scalar.copy(eye16[:], eye32[:])
            ones1t = cpool.tile([1, 128], F16, tag="ones1")      # maskneg row lhsT
            nc.scalar.copy(ones1t[:], ones32[0:1, :])
            onesKt = cpool.tile([1, 128], F32R, tag="onesK")     # wfac bcast lhsT
            nc.scalar.copy(onesKt[:], ones32[0:1, :])
            # onesblk[p, g] = 1 iff p // 36 == g  (band predicate 0 <= p-36g <= 35)
            onesblk = cpool.tile([IMG_GRP * LI, IMG_GRP], F32, tag="ob32")
            nc.gpsimd.memset(onesblk[:], 1.0)
            nc.gpsimd.affine_select(
                onesblk[:], onesblk[:], pattern=[[-LI, IMG_GRP]],
                compare_op=mybir.AluOpType.is_ge, fill=0.0, base=0, channel_multiplier=1,
            )
            nc.gpsimd.affine_select(
                onesblk[:], onesblk[:], pattern=[[LI, IMG_GRP]],
                compare_op=mybir.AluOpType.is_ge, fill=0.0, base=LI - 1, channel_multiplier=-1,
            )
            onesblkt = cpool.tile([IMG_GRP * LI, IMG_GRP], F32R, tag="ob")
            nc.scalar.copy(onesblkt[:], onesblk[:])
            # onesblkT[g, k] = 1 iff k // 36 == g; gmask = onesblkT^T @ onesblkT
            onesblkT = cpool.tile([IMG_GRP, IMG_GRP * LI], F32, tag="obT")
            nc.gpsimd.memset(onesblkT[:], 1.0)
            nc.gpsimd.affine_select(
                onesblkT[:], onesblkT[:], pattern=[[1, IMG_GRP * LI]],
                compare_op=mybir.AluOpType.is_ge, fill=0.0, base=0, channel_multiplier=-LI,
            )
            nc.gpsimd.affine_select(
                onesblkT[:], onesblkT[:], pattern=[[-1, IMG_GRP * LI]],
                compare_op=mybir.AluOpType.is_ge, fill=0.0, base=LI - 1, channel_multiplier=LI,
            )
            gmask_ps = pc.tile([IMG_GRP * LI, IMG_GRP * LI], F32, tag="cs")
            nc.tensor.matmul(gmask_ps[:], onesblkT[:], onesblkT[:], start=True, stop=True)
            gmask32 = cpool.tile([IMG_GRP * LI, IMG_GRP * LI], F32, tag="gmask")
            nc.scalar.copy(gmask32[:], gmask_ps[:])

            masknegt = cpool.tile([1, WF], F16, tag="mn")
            nc.sync.dma_start(masknegt[:], combo[IMROWS + SROWS : IMROWS + SROWS + 1, 0:WF])
            wdl16 = cpool.tile([1, WF], F16, tag="wdl16")
            nc.sync.dma_start(wdl16[:], combo[IMROWS + SROWS + 1 : IMROWS + SROWS + 2, 0:WF])
            wdlt = cpool.tile([1, WF], F32, tag="wdl")
            nc.scalar.copy(wdlt[:], wdl16[:])

            # ---- transpose this core's im slice, AllGather over NeuronLink ---
            imT_loc = cpool.tile([128, 8, IMROWS], F16, tag="imTloc")
            for t in range((IMROWS + 127) // 128):  # 576 = 4*128 + 64
                p0 = t * 128
                pn = min(128, IMROWS - p0)
                natt = natp.tile([128, D], F16, tag="nat")
                nc.sync.dma_start(natt[0:pn, :], combo[p0 : p0 + pn, :])
                for c in range(8):
                    tp = pc.tile([128, 128], F16, tag="cs")
                    nc.tensor.transpose(
                        tp[:, 0:pn], natt[0:pn, c * 128 : (c + 1) * 128], eye16[0:pn, 0:pn]
                    )
                    nc.scalar.copy(imT_loc[:, c, p0 : p0 + pn], tp[:, 0:pn])

            ag_in = dram.tile([128, 8 * IMROWS], F16)
            ag_out = dram.tile([NC, 128, 8 * IMROWS], F16, addr_space="Shared")
            nc.sync.dma_start(ag_in[:], imT_loc[:].rearrange("p c k -> p (c k)"))
            nc.gpsimd.collective_compute(
                "AllGather",
                mybir.AluOpType.bypass,
                replica_groups=[list(range(NC))],
                ins=[ag_in.opt()],
                outs=[ag_out.opt()],
            )

            # imT8[d_lo, c, seg*576 + k] = ag_out[seg, d_lo, c*576 + k]
            imT8 = cpool.tile([128, 8, B * LI], F16, tag="imT8")
            for seg in range(NC):
                for c in range(8):
                    nc.sync.dma_start(
                        imT8[:, c, seg * IMROWS : (seg + 1) * IMROWS],
                        ag_out[seg, :, c * IMROWS : (c + 1) * IMROWS],
                    )

            # ---- transpose this core's caption slice -------------------------
            sT8 = cpool.tile([128, 8, WF], F16, tag="sT8")
            for t in range((SROWS + 127) // 128):  # 800 = 6*128 + 32
                p0 = t * 128
                pn = min(128, SROWS - p0)
                natt = natp.tile([128, D], F16, tag="nat")
                nc.sync.dma_start(natt[0:pn, :], combo[IMROWS + p0 : IMROWS + p0 + pn, :])
                for c in range(8):
                    tp = pc.tile([128, 128], F16, tag="cs")
                    nc.tensor.transpose(
                        tp[:, 0:pn], natt[0:pn, c * 128 : (c + 1) * 128], eye16[0:pn, 0:pn]
                    )
                    nc.scalar.copy(sT8[:, c, p0 : p0 + pn], tp[:, 0:pn])

            # ---- caption word norms -> wfac row -> broadcast to 128 rows -----
            # capn2[w] = sum_d s[w,d]^2 via ones^T @ (sT^2), accumulated over
            # the 8 D-chunks.
            capn2_ps = pc.tile([1, WF], F32, tag="cs")
            for c in range(8):
                sq = work.tile([128, WF], F32, tag="e")
                nc.scalar.activation(sq[:], sT8[:, c, :], mybir.ActivationFunctionType.Square)
                for n0, n1 in NCH:
                    nc.tensor.matmul(
                        capn2_ps[:, n0:n1], ones32[:, 0:1], sq[:, n0:n1],
                        start=(c == 0), stop=(c == 7),
                    )
            # wfac = (wmask/len) / sqrt(capn2)
            wf0 = small.tile([1, WF], F32, tag="wf0")
            nc.scalar.sqrt(wf0[:], capn2_ps[:])
            wf1 = small.tile([1, WF], F32, tag="wf1")
            nc.vector.reciprocal(wf1[:], wf0[:])
            wfacv = small.tile([1, WF], F32, tag="wfacv")
            nc.vector.tensor_tensor(wfacv[:], wf1[:], wdlt[:], op=mybir.AluOpType.mult)
            wfacr = small.tile([1, WF], F32R, tag="wfacr")
            nc.scalar.copy(wfacr[:], wfacv[:])
            bc_ps = pc.tile([128, WF], F32, tag="cs")
            for n0, n1 in NCH:
                nc.tensor.matmul(bc_ps[:, n0:n1], onesKt[:], wfacr[:, n0:n1], start=True, stop=True)
            wfacb = cpool.tile([128, WF], F32, tag="wfacb")
            nc.scalar.copy(wfacb[:], bc_ps[:])

            nst = stage.tile([128, WF], F32, tag="nst")
            wst = stage.tile([128, WF], F32, tag="wst")

            # ---- main loop over image groups ---------------------------------
            for b in range(NB):
                ng = min(IMG_GRP, B - b * IMG_GRP)   # images in this group
                P = ng * LI                          # partitions used
                goff = b * IMG_GRP * LI

                # A[P, WF] = im_g @ s^T (+ word-mask row), fp16 PE, f32 PSUM
                a_ps = pa.tile([P, WF], F32, tag="AT")
                for n0, n1 in NCH:
                    for c in range(8):
                        nc.tensor.matmul(
                            a_ps[:, n0:n1], imT8[:, c, goff : goff + P], sT8[:, c, n0:n1],
                            start=(c == 0), stop=False,
                        )
                    nc.tensor.matmul(
                        a_ps[:, n0:n1], ones1t[0:1, 0:P], masknegt[0:1, n0:n1],
                        start=False, stop=True,
                    )

                # block-diagonal Gram of this group's images (zeroed cross terms)
                g_ps = pc.tile([IMG_GRP * LI, IMG_GRP * LI], F32, tag="cs")
                for c in range(8):
                    nc.tensor.matmul(
                        g_ps[0:P, 0:P], imT8[:, c, goff : goff + P], imT8[:, c, goff : goff + P],
                        start=(c == 0), stop=(c == 7),
                    )
                g16 = small.tile([IMG_GRP * LI, IMG_GRP * LI], F16, tag="g16")
                nc.vector.tensor_tensor(g16[0:P, 0:P], g_ps[0:P, 0:P], gmask32[0:P, 0:P], op=mybir.AluOpType.mult)

                am = work.tile([P, WF], F32, tag="am")
                nc.scalar.copy(am[:], a_ps[:])
                # word softmax with per-(row, caption) max subtraction
                mx = small.tile([P, CAP], F32, tag="mx")
                nc.vector.tensor_reduce(
                    mx[:], a_ps[:].rearrange("p (c w) -> p c w", c=CAP, w=LW),
                    axis=mybir.AxisListType.X, op=mybir.AluOpType.max,
                )
                sub = work.tile([P, WF], F32, tag="sub")
                nc.gpsimd.tensor_tensor(
                    sub[:].rearrange("p (c w) -> p c w", c=CAP, w=LW),
                    am[:].rearrange("p (c w) -> p c w", c=CAP, w=LW),
                    mx[:].unsqueeze(2).broadcast_to([P, CAP, LW]),
                    op=mybir.AluOpType.subtract,
                )
                e = work.tile([P, WF], F32, tag="e")
                nc.scalar.activation(e[:], sub[:], mybir.ActivationFunctionType.Exp)
                z = small.tile([P, CAP], F32, tag="z")
                nc.vector.tensor_reduce(
                    z[:], e[:].rearrange("p (c w) -> p c w", c=CAP, w=LW),
                    axis=mybir.AxisListType.X, op=mybir.AluOpType.add,
                )
                rz = small.tile([P, CAP], F32, tag="rz")
                nc.vector.reciprocal(rz[:], z[:])
                m = work.tile([P, WF], F32, tag="m")
                nc.vector.tensor_tensor(
                    m[:].rearrange("p (c w) -> p c w", c=CAP, w=LW),
                    e[:].rearrange("p (c w) -> p c w", c=CAP, w=LW),
                    rz[:].unsqueeze(2).broadcast_to([P, CAP, LW]),
                    op=mybir.AluOpType.mult,
                )
                # E2 = exp(lam * a1): fp16 copy feeds the PE, f32r copy the DVE
                e2h = work.tile([P, WF], F16, tag="e2h")
                nc.scalar.activation(e2h[:], m[:], mybir.ActivationFunctionType.Exp, bias=0.0, scale=LAM)
                e2f = work.tile([P, WF], F32R, tag="e2f")
                nc.scalar.activation(e2f[:], m[:], mybir.ActivationFunctionType.Exp, bias=0.0, scale=LAM)

                f = work.tile([P, WF], F32R, tag="f")
                nc.gpsimd.tensor_tensor(f[:], am[:], e2f[:], op=mybir.AluOpType.mult)

                t_ps = pa.tile([P, WF], F32, tag="AT")
                for n0, n1 in NCH:
                    nc.tensor.matmul(t_ps[:, n0:n1], g16[0:P, 0:P], e2h[:, n0:n1], start=True, stop=True)
                u = work.tile([P, WF], F32R, tag="u")
                nc.vector.tensor_tensor(u[:], t_ps[:], e2f[:], op=mybir.AluOpType.mult)

                n_ps = pc.tile([IMG_GRP, WF], F32, tag="cs")
                for n0, n1 in NCH:
                    nc.tensor.matmul(n_ps[0:ng, n0:n1], onesblkt[0:P, 0:ng], f[:, n0:n1], start=True, stop=True)
                w_ps = pc.tile([IMG_GRP, WF], F32, tag="cs")
                for n0, n1 in NCH:
                    nc.tensor.matmul(w_ps[0:ng, n0:n1], onesblkt[0:P, 0:ng], u[:, n0:n1], start=True, stop=True)

                r0 = b * IMG_GRP
                nb_sb = small.tile([IMG_GRP, WF], F32, tag="nb_sb")
                wb_sb = small.tile([IMG_GRP, WF], F32, tag="wb_sb")
                nc.scalar.copy(nb_sb[0:ng, :], n_ps[0:ng, :])
                nc.scalar.copy(wb_sb[0:ng, :], w_ps[0:ng, :])
                nc.sync.dma_start(nst[r0 : r0 + ng, :], nb_sb[0:ng, :])
                nc.sync.dma_start(wst[r0 : r0 + ng, :], wb_sb[0:ng, :])

            # ---- finalize: scores block [128 images, 16 captions] ------------
            srt = work.tile([128, WF], F32, tag="am")
            nc.scalar.sqrt(srt[:], wst[:])
            q = work.tile([128, WF], F32, tag="e")
            nc.vector.tensor_tensor(q[:], nst[:], wfacb[:], op=mybir.AluOpType.mult)
            rsq = work.tile([128, WF], F32, tag="sub")
            nc.vector.reciprocal(rsq[:], srt[:])
            cosq = work.tile([128, WF], F32, tag="m")
            nc.vector.tensor_tensor(cosq[:], q[:], rsq[:], op=mybir.AluOpType.mult)
            sim = small.tile([128, CAP], F32, tag="sim")
            nc.vector.tensor_reduce(
                sim[:], cosq[:].rearrange("p (c w) -> p c w", c=CAP, w=LW),
                axis=mybir.AxisListType.X, op=mybir.AluOpType.add,
            )
            nc.sync.dma_start(scores_out[:], sim[:])

    return nc


# ---------------------------------------------------------------------------
# Host side
# ---------------------------------------------------------------------------
def _host_prep(im, s, s_l):
    """Build the single fp16 upload array, already concatenated across cores:
    [8 * COMBO_ROWS, D].  Rows per core: 576 im rows, 800 s rows, maskneg
    row, wmask/len row (aux rows live in cols 0:800)."""
    im = np.asarray(im, dtype=np.float32)
    s = np.asarray(s, dtype=np.float32)
    s_l = np.asarray(s_l).astype(np.int64)

    wmask = (np.arange(LW)[None, :] < s_l[:, None])              # [B, LW]
    maskneg_all = ((~wmask) * np.float32(MASKNEG)).astype(np.float16)
    wdl_all = (wmask / s_l[:, None].astype(np.float32)).astype(np.float16)

    combo = np.zeros((NC, COMBO_ROWS, D), dtype=np.float16)
    combo[:, 0:IMROWS, :] = im.reshape(NC, IMROWS, D)
    combo[:, IMROWS : IMROWS + SROWS, :] = s.reshape(NC, SROWS, D)
    combo[:, IMROWS + SROWS, 0:WF] = maskneg_all.reshape(NC, WF)
    combo[:, IMROWS + SROWS + 1, 0:WF] = wdl_all.reshape(NC, WF)
    return combo.reshape(NC * COMBO_ROWS, D)


def _host_loss(scores):
    """Exact hinge margin loss (max violation) on the full score matrix."""
    scores = scores.astype(np.float32)
    diag = np.diagonal(scores)
    cost_s = np.maximum(MARGIN + scores - diag[:, None], 0.0)
    cost_im = np.maximum(MARGIN + scores - diag[None, :], 0.0)
    np.fill_diagonal(cost_s, 0.0)
    np.fill_diagonal(cost_im, 0.0)
    return np.float32(cost_s.max(axis=1).sum() + cost_im.max(axis=0).sum())


def _make_runner(nc):
    """Persistent jitted SPMD executable (same mechanics as
    bass2jax.run_bass_via_pjrt, but built once and reused across calls)."""
    import jax
    from jax.sharding import Mesh, PartitionSpec
    try:
        from jax import shard_map
    except ImportError:
        from jax.experimental.shard_map import shard_map
    from concourse.bass2jax import _bass_exec_p, install_neuronx_cc_hook, partition_id_tensor

    install_neuronx_cc_hook()
    partition_name = nc.partition_id_tensor.name if nc.partition_id_tensor else None

    in_names, out_names, out_avals, out_shapes = [], [], [], []
    for alloc in nc.m.functions[0].allocations:
        if not isinstance(alloc, mybir.MemoryLocationSet):
            continue
        name = alloc.memorylocations[0].name
        if alloc.kind == "ExternalInput":
            if name != partition_name:
                in_names.append(name)
        elif alloc.kind == "ExternalOutput":
            shape = tuple(alloc.tensor_shape)
            dtype = mybir.dt.np(alloc.dtype)
            out_names.append(name)
            out_avals.append(jax.core.ShapedArray(shape, dtype))
            out_shapes.append((shape, dtype))
    n_params = len(in_names)
    n_outs = len(out_names)
    in_names_full = in_names + out_names
    if partition_name is not None:
        in_names_full.append(partition_name)
    donate = tuple(range(n_params, n_params + n_outs))

    def _body(*args):
        operands = list(args)
        if partition_name is not None:
            operands.append(partition_id_tensor())
        outs = _bass_exec_p.bind(
            *operands,
            out_avals=tuple(out_avals),
            in_names=tuple(in_names_full),
            out_names=tuple(out_names),
            lowering_input_output_aliases=(),
            sim_require_finite=True,
            sim_require_nnan=True,
            nc=nc,
        )
        return tuple(outs)

    devices = jax.devices()[:NC]
    assert len(devices) == NC
    mesh = Mesh(np.asarray(devices), ("core",))
    in_specs = (PartitionSpec("core"),) * (n_params + n_outs)
    out_specs = (PartitionSpec("core"),) * n_outs
    sharded = jax.jit(
        shard_map(_body, mesh=mesh, in_specs=in_specs, out_specs=out_specs, check_rep=False),
        donate_argnums=donate,
        keep_unused=True,
    )

    def call(global_in_map):
        ins = [np.ascontiguousarray(global_in_map[name]) for name in in_names]
        zeros = [np.zeros((NC * sh[0], *sh[1:]), dt) for sh, dt in out_shapes]
        outs = sharded(*ins, *zeros)
        return {
            name: np.asarray(outs[i]).reshape(NC, *out_shapes[i][0])
            for i, name in enumerate(out_names)
        }

    return call


class _Res:
    """Minimal stand-in for BassKernelResults (test.py reads exec_time_ns)."""
    exec_time_ns = None


def run(im, s, s_l, trace=False):
    """Returns (loss_scalar, scores[128,128], res)."""
    _install_patches()
    if "nc" not in _CACHE:
        _CACHE["nc"] = _build_program()
    nc = _CACHE["nc"]
    combo = _host_prep(im, s, s_l)

    out = None
    if "runner" not in _CACHE and not _CACHE.get("runner_failed"):
        try:
            _CACHE["runner"] = _make_runner(nc)
        except Exception:
            _CACHE["runner_failed"] = True
    if "runner" in _CACHE:
        try:
            out = _CACHE["runner"]({"combo": combo})
        except Exception:
            _CACHE.pop("runner", None)
            _CACHE["runner_failed"] = True
            out = None
    if out is None:
        # Fallback: stock per-call path.
        from concourse.bass_utils import run_bass_kernel_spmd
        combo_pc = combo.reshape(NC, COMBO_ROWS, D)
        res = run_bass_kernel_spmd(
            nc, [{"combo": combo_pc[c]} for c in range(NC)], list(range(NC)), trace=False
        )
        blocks = np.stack([res.results[c]["scores_out"] for c in range(NC)])
    else:
        blocks = out["scores_out"]                      # [NC, 128, CAP]

    scores = blocks.transpose(1, 0, 2).reshape(128, 128)
    loss = _host_loss(scores)
    return loss, scores, _Res()


def kernel(im, s, s_l):
    loss, _, _ = run(im, s, s_l)
    return np.array(loss, dtype=np.float32)


# revision 10
# speedup vs baseline: 8.0733x; 1.5762x over previous
"""Trainium2 Bass kernel for nn_ContrastiveLoss (stacked cross-attention t2i).

Strategy (8 NeuronCores, caption-sharded; transfer-optimized):
  The axon loopback relay moves host<->device data at only ~38 MB/s, so the
  end-to-end time is dominated by upload bytes.  This version uploads one
  fp16 array per core (~2.8 MB: its 16 images + its 16 captions + two aux
  rows) instead of the ~25 MB fp32 the old kernel shipped:
    - im is uploaded SHARDED (1/8 per core) and AllGathered on-device over
      NeuronLink.
    - s stays sharded (each core only scores its own 16 captions).
    - transposes (im, s -> D-major), per-image Gram matrices, caption-word
      norms, and the wfac broadcast are all computed on device.
    - A- and T-matmuls run in fp16 (PSUM accumulates fp32); the softmax /
      cosine chain stays fp32, identical in structure to the reference.
  Each core returns its (128 x 16) score columns; the host assembles the
  full 128x128 score matrix and computes the (trivial) hinge margin loss.

Math note: with E2 = exp(lam * a1) (unnormalized region attention),
  cos = (sum_r E2*A) / (cap_n * sqrt(E2^T G E2)) exactly, because the region
softmax normalizer cancels between numerator and |weighted context|.
"""

import numpy as np

import concourse.bass as bass
import concourse.tile as tile
from concourse import mybir
from concourse.vector_clock import ScopedClock

# ---------------------------------------------------------------------------
# Workaround for this toolchain: walrus rejects instructions carrying more
# than one semaphore wait.  Split extra waits onto standalone EventSemaphore
# instructions (the same thing wait_ge emits) just before the offender.
# ---------------------------------------------------------------------------
_PATCHED = False


def _install_patches():
    global _PATCHED
    if _PATCHED:
        return
    _PATCHED = True

    def _drain_and_barrier(self, tick_clock, wait_clock):
        nc = self.nc
        drain_inst = nc.sync.drain()
        wait_clock.add_sem_waits(
            drain_inst.ins, ScopedClock({None: tick_clock.global_clock})
        )
        waits = list(drain_inst.ins.sync_info.on_wait)
        if len(waits) > 1:
            drain_inst.ins.sync_info.on_wait = waits[:1]
            for w in waits[1:]:
                extra = nc.sync.drain()
                extra.ins.sync_info = mybir.SyncInfo(on_wait=[w], on_update=[])
        nc.all_engine_barrier()
        popped = nc._tile_sem_poison_stack.pop()
        assert popped is self._sem_poison
        nc.clear_and_free_semaphores(list(self.sems.allocated().values()))
        nc.all_engine_barrier()

    tile.TileContext._drain_and_barrier = _drain_and_barrier

    import concourse.bass_utils as bass_utils
    import concourse.bass2jax as bass2jax
    import orjson

    _orig_compile = bass_utils.compile_bir_kernel

    def _split_waits_in_bir(bir_json: bytes) -> bytes:
        m = orjson.loads(bir_json)
        for fn in m.get("functions", []):
            for blk in fn.get("blocks", []):
                insts = blk.get("instructions", [])
                new_insts = []
                for ins in insts:
                    si = ins.get("sync_info")
                    waits = (si or {}).get("on_wait") or []
                    if len(waits) > 1:
                        for k, w in enumerate(waits[:-1]):
                            new_insts.append(
                                {
                                    "name": f"{ins['name']}_wsplit{k}",
                                    "opcode": "EventSemaphore",
                                    "engine": ins["engine"],
                                    "ins": [],
                                    "outs": [],
                                    "debug": ins.get("debug"),
                                    "sync_info": {"on_update": [], "on_wait": [w]},
                                }
                            )
                        si["on_wait"] = waits[-1:]
                    new_insts.append(ins)
                blk["instructions"] = new_insts
        return orjson.dumps(m)

    def _patched_compile(bir_json, tmpdir, neff_name="file.neff"):
        return _orig_compile(_split_waits_in_bir(bir_json), tmpdir, neff_name)

    bass_utils.compile_bir_kernel = _patched_compile
    bass2jax.compile_bir_kernel = _patched_compile


# ---------------------------------------------------------------------------
# Problem constants (hardcoded per the task contract).
# ---------------------------------------------------------------------------
B = 128           # images == captions
LI = 36           # image regions
LW = 50           # padded caption words
D = 1024          # feature dim
NC = 8            # cores
CAP = B // NC     # captions per core (16)
WF = CAP * LW     # free width of the batched tiles (800)
IMG_GRP = 3       # images per batch
NB = (B + IMG_GRP - 1) // IMG_GRP  # 43 batches (42x3 + 1x2)
IMROWS = CAP * LI         # 576 rows of this core's images
SROWS = CAP * LW          # 800 rows of this core's captions
COMBO_ROWS = IMROWS + SROWS + 2   # + maskneg row + wdl row
LAM = 9.0
MARGIN = 0.2
MASKNEG = -30000.0

F32 = mybir.dt.float32
F32R = mybir.dt.float32r
F16 = mybir.dt.float16

NCH = [(0, 512), (512, WF)]

_CACHE = {}


def _build_program():
    nc = bass.Bass("TRN2", target_bir_lowering=False, debug=False, num_devices=NC)

    combo = nc.dram_tensor("combo", [COMBO_ROWS, D], F16, kind="ExternalInput")
    scores_out = nc.dram_tensor("scores_out", [128, CAP], F32, kind="ExternalOutput")

    with tile.TileContext(nc) as tc:
        with (
            tc.tile_pool(name="const", bufs=1) as cpool,
            tc.tile_pool(name="nat", bufs=2) as natp,
            tc.tile_pool(name="work", bufs=2) as work,
            tc.tile_pool(name="small", bufs=2) as small,
            tc.tile_pool(name="stage", bufs=1) as stage,
            tc.tile_pool(name="pa", bufs=2, space="PSUM") as pa,
            tc.tile_pool(name="pc", bufs=2, space="PSUM") as pc,
            tc.tile_pool(name="dram", bufs=1, space="DRAM") as dram,
        ):
            # ---- on-device constants -----------------------------------------
            ones32 = cpool.tile([128, 128], F32, tag="ones32")
            nc.vector.memset(ones32[:], 1.0)
            eye32 = cpool.tile([128, 128], F32, tag="eye32")
            nc.gpsimd.affine_select(
                eye32[:], ones32[:], pattern=[[-1, 128]],
                compare_op=mybir.AluOpType.is_equal, fill=0.0,
                base=0, channel_multiplier=1,
            )
            eye16 = cpool.tile([128, 128], F16, tag="eye16")
            nc.scalar.copy(eye16[:], eye32[:])
            ones1t = cpool.tile([1, 128], F16, tag="ones1")      # maskneg row lhsT
            nc.scalar.copy(ones1t[:], ones32[0:1, :])
            onesKt = cpool.tile([1, 128], F32R, tag="onesK")     # wfac bcast lhsT
            nc.scalar.copy(onesKt[:], ones32[0:1, :])
            # onesblk[p, g] = 1 iff p // 36 == g  (band predicate 0 <= p-36g <= 35)
            onesblk = cpool.tile([IMG_GRP * LI, IMG_GRP], F32, tag="ob32")
            nc.gpsimd.memset(onesblk[:], 1.0)
            nc.gpsimd.affine_select(
                onesblk[:], onesblk[:], pattern=[[-LI, IMG_GRP]],
                compare_op=mybir.AluOpType.is_ge, fill=0.0, base=0, channel_multiplier=1,
            )
            nc.gpsimd.affine_select(
                onesblk[:], onesblk[:], pattern=[[LI, IMG_GRP]],
                compare_op=mybir.AluOpType.is_ge, fill=0.0, base=LI - 1, channel_multiplier=-1,
            )
            onesblkt = cpool.tile([IMG_GRP * LI, IMG_GRP], F32R, tag="ob")
            nc.scalar.copy(onesblkt[:], onesblk[:])
            # onesblkT[g, k] = 1 iff k // 36 == g; gmask = onesblkT^T @ onesblkT
            onesblkT = cpool.tile([IMG_GRP, IMG_GRP * LI], F32, tag="obT")
            nc.gpsimd.memset(onesblkT[:], 1.0)
            nc.gpsimd.affine_select(
                onesblkT[:], onesblkT[:], pattern=[[1, IMG_GRP * LI]],
                compare_op=mybir.AluOpType.is_ge, fill=0.0, base=0, channel_multiplier=-LI,
            )
            nc.gpsimd.affine_select(
                onesblkT[:], onesblkT[:], pattern=[[-1, IMG_GRP * LI]],
                compare_op=mybir.AluOpType.is_ge, fill=0.0, base=LI - 1, channel_multiplier=LI,
            )
            gmask_ps = pc.tile([IMG_GRP * LI, IMG_GRP * LI], F32, tag="cs")
            nc.tensor.matmul(gmask_ps[:], onesblkT[:], onesblkT[:], start=True, stop=True)
            gmask32 = cpool.tile([IMG_GRP * LI, IMG_GRP * LI], F32, tag="gmask")
            nc.scalar.copy(gmask32[:], gmask_ps[:])

            masknegt = cpool.tile([1, WF], F16, tag="mn")
            nc.sync.dma_start(masknegt[:], combo[IMROWS + SROWS : IMROWS + SROWS + 1, 0:WF])
            wdl16 = cpool.tile([1, WF], F16, tag="wdl16")
            nc.sync.dma_start(wdl16[:], combo[IMROWS + SROWS + 1 : IMROWS + SROWS + 2, 0:WF])
            wdlt = cpool.tile([1, WF], F32, tag="wdl")
            nc.scalar.copy(wdlt[:], wdl16[:])

            # ---- transpose this core's im slice, AllGather over NeuronLink ---
            imT_loc = cpool.tile([128, 8, IMROWS], F16, tag="imTloc")
            for t in range((IMROWS + 127) // 128):  # 576 = 4*128 + 64
                p0 = t * 128
                pn = min(128, IMROWS - p0)
                natt = natp.tile([128, D], F16, tag="nat")
                nc.sync.dma_start(natt[0:pn, :], combo[p0 : p0 + pn, :])
                for c in range(8):
                    tp = pc.tile([128, 128], F16, tag="cs")
                    nc.tensor.transpose(
                        tp[:, 0:pn], natt[0:pn, c * 128 : (c + 1) * 128], eye16[0:pn, 0:pn]
                    )
                    nc.scalar.copy(imT_loc[:, c, p0 : p0 + pn], tp[:, 0:pn])

            ag_in = dram.tile([128, 8 * IMROWS], F16)
            ag_out = dram.tile([NC, 128, 8 * IMROWS], F16, addr_space="Shared")
            nc.sync.dma_start(ag_in[:], imT_loc[:].rearrange("p c k -> p (c k)"))
            nc.gpsimd.collective_compute(
                "AllGather",
                mybir.AluOpType.bypass,
                replica_groups=[list(range(NC))],
                ins=[ag_in.opt()],
                outs=[ag_out.opt()],
            )

            # imT8[d_lo, c, seg*576 + k] = ag_out[seg, d_lo, c*576 + k]
            imT8 = cpool.tile([128, 8, B * LI], F16, tag="imT8")
            for seg in range(NC):
                for c in range(8):
                    nc.sync.dma_start(
                        imT8[:, c, seg * IMROWS : (seg + 1) * IMROWS],
                        ag_out[seg, :, c * IMROWS : (c + 1) * IMROWS],
                    )

            # ---- transpose this core's caption slice -------------------------
            sT8 = cpool.tile([128, 8, WF], F16, tag="sT8")
            for t in range((SROWS + 127) // 128):  # 800 = 6*128 + 32
                p0 = t * 128
                pn = min(128, SROWS - p0)
                natt = natp.tile([128, D], F16, tag="nat")
                nc.sync.dma_start(natt[0:pn, :], combo[IMROWS + p0 : IMROWS + p0 + pn, :])
                for c in range(8):
                    tp = pc.tile([128, 128], F16, tag="cs")
                    nc.tensor.transpose(
                        tp[:, 0:pn], natt[0:pn, c * 128 : (c + 1) * 128], eye16[0:pn, 0:pn]
                    )
                    nc.scalar.copy(sT8[:, c, p0 : p0 + pn], tp[:, 0:pn])

            # ---- caption word norms -> wfac row -> broadcast to 128 rows -----
            # capn2[w] = sum_d s[w,d]^2 via ones^T @ (sT^2), accumulated over
            # the 8 D-chunks.
            capn2_ps = pc.tile([1, WF], F32, tag="cs")
            for c in range(8):
                sq = work.tile([128, WF], F32, tag="e")
                nc.scalar.activation(sq[:], sT8[:, c, :], mybir.ActivationFunctionType.Square)
                for n0, n1 in NCH:
                    nc.tensor.matmul(
                        capn2_ps[:, n0:n1], ones32[:, 0:1], sq[:, n0:n1],
                        start=(c == 0), stop=(c == 7),
                    )
            # wfac = (wmask/len) / sqrt(capn2)
            wf0 = small.tile([1, WF], F32, tag="wf0")
            nc.scalar.sqrt(wf0[:], capn2_ps[:])
            wf1 = small.tile([1, WF], F32, tag="wf1")
            nc.vector.reciprocal(wf1[:], wf0[:])
            wfacv = small.tile([1, WF], F32, tag="wfacv")
            nc.vector.tensor_tensor(wfacv[:], wf1[:], wdlt[:], op=mybir.AluOpType.mult)
            wfacr = small.tile([1, WF], F32R, tag="wfacr")
            nc.scalar.copy(wfacr[:], wfacv[:])
            bc_ps = pc.tile([128, WF], F32, tag="cs")
            for n0, n1 in NCH:
                nc.tensor.matmul(bc_ps[:, n0:n1], onesKt[:], wfacr[:, n0:n1], start=True, stop=True)
            wfacb = cpool.tile([128, WF], F32, tag="wfacb")
            nc.scalar.copy(wfacb[:], bc_ps[:])

            nst = stage.tile([128, WF], F32, tag="nst")
            wst = stage.tile([128, WF], F32, tag="wst")

            # ---- main loop over image groups ---------------------------------
            for b in range(NB):
                ng = min(IMG_GRP, B - b * IMG_GRP)   # images in this group
                P = ng * LI                          # partitions used
                goff = b * IMG_GRP * LI

                # A[P, WF] = im_g @ s^T (+ word-mask row), fp16 PE, f32 PSUM
                a_ps = pa.tile([P, WF], F32, tag="AT")
                for n0, n1 in NCH:
                    for c in range(8):
                        nc.tensor.matmul(
                            a_ps[:, n0:n1], imT8[:, c, goff : goff + P], sT8[:, c, n0:n1],
                            start=(c == 0), stop=False,
                        )
                    nc.tensor.matmul(
                        a_ps[:, n0:n1], ones1t[0:1, 0:P], masknegt[0:1, n0:n1],
                        start=False, stop=True,
                    )

                # block-diagonal Gram of this group's images (zeroed cross terms)
                g_ps = pc.tile([IMG_GRP * LI, IMG_GRP * LI], F32, tag="cs")
                for c in range(8):
                    nc.tensor.matmul(
                        g_ps[0:P, 0:P], imT8[:, c, goff : goff + P], imT8[:, c, goff : goff + P],
                        start=(c == 0), stop=(c == 7),
                    )
                g16 = small.tile([IMG_GRP * LI, IMG_GRP * LI], F16, tag="g16")
                nc.vector.tensor_tensor(g16[0:P, 0:P], g_ps[0:P, 0:P], gmask32[0:P, 0:P], op=mybir.AluOpType.mult)

                am = work.tile([P, WF], F32, tag="am")
                nc.scalar.copy(am[:], a_ps[:])
                # word softmax with per-(row, caption) max subtraction
                mx = small.tile([P, CAP], F32, tag="mx")
                nc.vector.tensor_reduce(
                    mx[:], a_ps[:].rearrange("p (c w) -> p c w", c=CAP, w=LW),
                    axis=mybir.AxisListType.X, op=mybir.AluOpType.max,
                )
                sub = work.tile([P, WF], F32, tag="sub")
                nc.gpsimd.tensor_tensor(
                    sub[:].rearrange("p (c w) -> p c w", c=CAP, w=LW),
                    am[:].rearrange("p (c w) -> p c w", c=CAP, w=LW),
                    mx[:].unsqueeze(2).broadcast_to([P, CAP, LW]),
                    op=mybir.AluOpType.subtract,
                )
                e = work.tile([P, WF], F32, tag="e")
                nc.scalar.activation(e[:], sub[:], mybir.ActivationFunctionType.Exp)
                z = small.tile([P, CAP], F32, tag="z")
                nc.vector.tensor_reduce(
                    z[:], e[:].rearrange("p (c w) -> p c w", c=CAP, w=LW),
                    axis=mybir.AxisListType.X, op=mybir.AluOpType.add,
                )
                rz = small.tile([P, CAP], F32, tag="rz")
                nc.vector.reciprocal(rz[:], z[:])
                m = work.tile([P, WF], F32, tag="m")
                nc.vector.tensor_tensor(
                    m[:].rearrange("p (c w) -> p c w", c=CAP, w=LW),
                    e[:].rearrange("p (c w) -> p c w", c=CAP, w=LW),
                    rz[:].unsqueeze(2).broadcast_to([P, CAP, LW]),
                    op=mybir.AluOpType.mult,
                )
                # E2 = exp(lam * a1): fp16 copy feeds the PE, f32r copy the DVE
                e2h = work.tile([P, WF], F16, tag="e2h")
                nc.scalar.activation(e2h[:], m[:], mybir.ActivationFunctionType.Exp, bias=0.0, scale=LAM)
                e2f = work.tile([P, WF], F32R, tag="e2f")
                nc.scalar.activation(e2f[:], m[:], mybir.ActivationFunctionType.Exp, bias=0.0, scale=LAM)

                f = work.tile([P, WF], F32R, tag="f")
                nc.gpsimd.tensor_tensor(f[:], am[:], e2f[:], op=mybir.AluOpType.mult)

                t_ps = pa.tile([P, WF], F32, tag="AT")
                for n0, n1 in NCH:
                    nc.tensor.matmul(t_ps[:, n0:n1], g16[0:P, 0:P], e2h[:, n0:n1], start=True, stop=True)
                u = work.tile([P, WF], F32R, tag="u")
                nc.vector.tensor_tensor(u[:], t_ps[:], e2f[:], op=mybir.AluOpType.mult)

                n_ps = pc.tile([IMG_GRP, WF], F32, tag="cs")
                for n0, n1 in NCH:
                    nc.tensor.matmul(n_ps[0:ng, n0:n1], onesblkt[0:P, 0:ng], f[:, n0:n1], start=True, stop=True)
                w_ps = pc.tile([IMG_GRP, WF], F32, tag="cs")
                for n0, n1 in NCH:
                    nc.tensor.matmul(w_ps[0:ng, n0:n1], onesblkt[0:P, 0:ng], u[:, n0:n1], start=True, stop=True)

                r0 = b * IMG_GRP
                nb_sb = small.tile([IMG_GRP, WF], F32, tag="nb_sb")
                wb_sb = small.tile([IMG_GRP, WF], F32, tag="wb_sb")
                nc.scalar.copy(nb_sb[0:ng, :], n_ps[0:ng, :])
                nc.scalar.copy(wb_sb[0:ng, :], w_ps[0:ng, :])
                nc.sync.dma_start(nst[r0 : r0 + ng, :], nb_sb[0:ng, :])
                nc.sync.dma_start(wst[r0 : r0 + ng, :], wb_sb[0:ng, :])

            # ---- finalize: scores block [128 images, 16 captions] ------------
            srt = work.tile([128, WF], F32, tag="am")
            nc.scalar.sqrt(srt[:], wst[:])
            q = work.tile([128, WF], F32, tag="e")
            nc.vector.tensor_tensor(q[:], nst[:], wfacb[:], op=mybir.AluOpType.mult)
            rsq = work.tile([128, WF], F32, tag="sub")
            nc.vector.reciprocal(rsq[:], srt[:])
            cosq = work.tile([128, WF], F32, tag="m")
            nc.vector.tensor_tensor(cosq[:], q[:], rsq[:], op=mybir.AluOpType.mult)
            sim = small.tile([128, CAP], F32, tag="sim")
            nc.vector.tensor_reduce(
                sim[:], cosq[:].rearrange("p (c w) -> p c w", c=CAP, w=LW),
                axis=mybir.AxisListType.X, op=mybir.AluOpType.add,
            )
            nc.sync.dma_start(scores_out[:], sim[:])

    return nc


# ---------------------------------------------------------------------------
# Host side
# ---------------------------------------------------------------------------
def _host_prep(im, s, s_l):
    """Build the single fp16 upload array, already concatenated across cores:
    [8 * COMBO_ROWS, D].  Rows per core: 576 im rows, 800 s rows, maskneg
    row, wmask/len row (aux rows live in cols 0:800)."""
    im = np.asarray(im, dtype=np.float32)
    s = np.asarray(s, dtype=np.float32)
    s_l = np.asarray(s_l).astype(np.int64)

    wmask = (np.arange(LW)[None, :] < s_l[:, None])              # [B, LW]
    maskneg_all = ((~wmask) * np.float32(MASKNEG)).astype(np.float16)
    wdl_all = (wmask / s_l[:, None].astype(np.float32)).astype(np.float16)

    combo = np.zeros((NC, COMBO_ROWS, D), dtype=np.float16)
    combo[:, 0:IMROWS, :] = im.reshape(NC, IMROWS, D)
    combo[:, IMROWS : IMROWS + SROWS, :] = s.reshape(NC, SROWS, D)
    combo[:, IMROWS + SROWS, 0:WF] = maskneg_all.reshape(NC, WF)
    combo[:, IMROWS + SROWS + 1, 0:WF] = wdl_all.reshape(NC, WF)
    return combo.reshape(NC * COMBO_ROWS, D)


def _host_loss(scores):
    """Exact hinge margin loss (max violation) on the full score matrix."""
    scores = scores.astype(np.float32)
    diag = np.diagonal(scores)
    cost_s = np.maximum(MARGIN + scores - diag[:, None], 0.0)
    cost_im = np.maximum(MARGIN + scores - diag[None, :], 0.0)
    np.fill_diagonal(cost_s, 0.0)
    np.fill_diagonal(cost_im, 0.0)
    return np.float32(cost_s.max(axis=1).sum() + cost_im.max(axis=0).sum())


def _make_runner(nc):
    """Persistent jitted SPMD executable (same mechanics as
    bass2jax.run_bass_via_pjrt, but built once and reused across calls)."""
    import warnings
    import jax
    from jax.sharding import Mesh, PartitionSpec
    with warnings.catch_warnings():
        warnings.simplefilter("ignore")
        from jax.experimental.shard_map import shard_map
    from concourse.bass2jax import _bass_exec_p, install_neuronx_cc_hook, partition_id_tensor

    install_neuronx_cc_hook()
    partition_name = nc.partition_id_tensor.name if nc.partition_id_tensor else None

    in_names, out_names, out_avals, out_shapes = [], [], [], []
    for alloc in nc.m.functions[0].allocations:
        if not isinstance(alloc, mybir.MemoryLocationSet):
            continue
        name = alloc.memorylocations[0].name
        if alloc.kind == "ExternalInput":
            if name != partition_name:
                in_names.append(name)
        elif alloc.kind == "ExternalOutput":
            shape = tuple(alloc.tensor_shape)
            dtype = mybir.dt.np(alloc.dtype)
            out_names.append(name)
            out_avals.append(jax.core.ShapedArray(shape, dtype))
            out_shapes.append((shape, dtype))
    n_params = len(in_names)
    n_outs = len(out_names)
    in_names_full = in_names + out_names
    if partition_name is not None:
        in_names_full.append(partition_name)
    donate = tuple(range(n_params, n_params + n_outs))

    def _body(*args):
        operands = list(args)
        if partition_name is not None:
            operands.append(partition_id_tensor())
        outs = _bass_exec_p.bind(
            *operands,
            out_avals=tuple(out_avals),
            in_names=tuple(in_names_full),
            out_names=tuple(out_names),
            lowering_input_output_aliases=(),
            sim_require_finite=True,
            sim_require_nnan=True,
            nc=nc,
        )
        return tuple(outs)

    devices = jax.devices()[:NC]
    assert len(devices) == NC
    mesh = Mesh(np.asarray(devices), ("core",))
    in_specs = (PartitionSpec("core"),) * (n_params + n_outs)
    out_specs = (PartitionSpec("core"),) * n_outs
    sharded = jax.jit(
        shard_map(_body, mesh=mesh, in_specs=in_specs, out_specs=out_specs, check_rep=False),
        donate_argnums=donate,
        keep_unused=True,
    )

    def call(global_in_map):
        ins = [np.ascontiguousarray(global_in_map[name]) for name in in_names]
        zeros = [np.zeros((NC * sh[0], *sh[1:]), dt) for sh, dt in out_shapes]
        outs = sharded(*ins, *zeros)
        return {
            name: np.asarray(outs[i]).reshape(NC, *out_shapes[i][0])
            for i, name in enumerate(out_names)
        }

    return call


class _Res:
    """Minimal stand-in for BassKernelResults (test.py reads exec_time_ns)."""
    exec_time_ns = None


def run(im, s, s_l, trace=False):
    """Returns (loss_scalar, scores[128,128], res)."""
    _install_patches()
    if "nc" not in _CACHE:
        _CACHE["nc"] = _build_program()
    nc = _CACHE["nc"]
    combo = _host_prep(im, s, s_l)

    out = None
    if "runner" not in _CACHE and not _CACHE.get("runner_failed"):
        try:
            _CACHE["runner"] = _make_runner(nc)
        except Exception:
            _CACHE["runner_failed"] = True
    if "runner" in _CACHE:
        try:
            out = _CACHE["runner"]({"combo": combo})
        except Exception:
            _CACHE.pop("runner", None)
            _CACHE["runner_failed"] = True
            out = None
    if out is None:
        # Fallback: stock per-call path.
        from concourse.bass_utils import run_bass_kernel_spmd
        combo_pc = combo.reshape(NC, COMBO_ROWS, D)
        res = run_bass_kernel_spmd(
            nc, [{"combo": combo_pc[c]} for c in range(NC)], list(range(NC)), trace=False
        )
        blocks = np.stack([res.results[c]["scores_out"] for c in range(NC)])
    else:
        blocks = out["scores_out"]                      # [NC, 128, CAP]

    scores = blocks.transpose(1, 0, 2).reshape(128, 128)
    loss = _host_loss(scores)
    return loss, scores, _Res()


def kernel(im, s, s_l):
    loss, _, _ = run(im, s, s_l)
    return np.array(loss, dtype=np.float32)


# revision 21
# speedup vs baseline: 10.7517x; 1.3318x over previous
"""Trainium2 Bass kernel for nn_ContrastiveLoss (stacked cross-attention t2i).

Strategy (8 NeuronCores, caption-sharded; transfer-optimized):
  The axon loopback relay moves host<->device data at only ~38 MB/s, so the
  end-to-end time is dominated by upload bytes.  This version uploads one
  fp16 array per core (~2.8 MB: its 16 images + its 16 captions + two aux
  rows) instead of the ~25 MB fp32 the old kernel shipped:
    - im is uploaded SHARDED (1/8 per core) and AllGathered on-device over
      NeuronLink.
    - s stays sharded (each core only scores its own 16 captions).
    - transposes (im, s -> D-major), per-image Gram matrices, caption-word
      norms, and the wfac broadcast are all computed on device.
    - A- and T-matmuls run in fp16 (PSUM accumulates fp32); the softmax /
      cosine chain stays fp32, identical in structure to the reference.
  Each core returns its (128 x 16) score columns; the host assembles the
  full 128x128 score matrix and computes the (trivial) hinge margin loss.

Math note: with E2 = exp(lam * a1) (unnormalized region attention),
  cos = (sum_r E2*A) / (cap_n * sqrt(E2^T G E2)) exactly, because the region
softmax normalizer cancels between numerator and |weighted context|.
"""

import numpy as np

import concourse.bass as bass
import concourse.tile as tile
from concourse import mybir
from concourse.vector_clock import ScopedClock

# ---------------------------------------------------------------------------
# Workaround for this toolchain: walrus rejects instructions carrying more
# than one semaphore wait.  Split extra waits onto standalone EventSemaphore
# instructions (the same thing wait_ge emits) just before the offender.
# ---------------------------------------------------------------------------
_PATCHED = False


def _install_patches():
    global _PATCHED
    if _PATCHED:
        return
    _PATCHED = True

    def _drain_and_barrier(self, tick_clock, wait_clock):
        nc = self.nc
        drain_inst = nc.sync.drain()
        wait_clock.add_sem_waits(
            drain_inst.ins, ScopedClock({None: tick_clock.global_clock})
        )
        waits = list(drain_inst.ins.sync_info.on_wait)
        if len(waits) > 1:
            drain_inst.ins.sync_info.on_wait = waits[:1]
            for w in waits[1:]:
                extra = nc.sync.drain()
                extra.ins.sync_info = mybir.SyncInfo(on_wait=[w], on_update=[])
        nc.all_engine_barrier()
        popped = nc._tile_sem_poison_stack.pop()
        assert popped is self._sem_poison
        nc.clear_and_free_semaphores(list(self.sems.allocated().values()))
        nc.all_engine_barrier()

    tile.TileContext._drain_and_barrier = _drain_and_barrier

    import concourse.bass_utils as bass_utils
    import concourse.bass2jax as bass2jax
    import orjson

    _orig_compile = bass_utils.compile_bir_kernel

    def _split_waits_in_bir(bir_json: bytes) -> bytes:
        m = orjson.loads(bir_json)
        for fn in m.get("functions", []):
            for blk in fn.get("blocks", []):
                insts = blk.get("instructions", [])
                new_insts = []
                for ins in insts:
                    si = ins.get("sync_info")
                    waits = (si or {}).get("on_wait") or []
                    if len(waits) > 1:
                        for k, w in enumerate(waits[:-1]):
                            new_insts.append(
                                {
                                    "name": f"{ins['name']}_wsplit{k}",
                                    "opcode": "EventSemaphore",
                                    "engine": ins["engine"],
                                    "ins": [],
                                    "outs": [],
                                    "debug": ins.get("debug"),
                                    "sync_info": {"on_update": [], "on_wait": [w]},
                                }
                            )
                        si["on_wait"] = waits[-1:]
                    new_insts.append(ins)
                blk["instructions"] = new_insts
        return orjson.dumps(m)

    def _patched_compile(bir_json, tmpdir, neff_name="file.neff"):
        return _orig_compile(_split_waits_in_bir(bir_json), tmpdir, neff_name)

    bass_utils.compile_bir_kernel = _patched_compile
    bass2jax.compile_bir_kernel = _patched_compile


# ---------------------------------------------------------------------------
# Problem constants (hardcoded per the task contract).
# ---------------------------------------------------------------------------
B = 128           # images == captions
LI = 36           # image regions
LW = 50           # padded caption words
D = 1024          # feature dim
NC = 8            # cores
CAP = B // NC     # captions per core (16)
WF = CAP * LW     # free width of the batched tiles (800)
IMG_GRP = 3       # images per batch
NB = (B + IMG_GRP - 1) // IMG_GRP  # 43 batches (42x3 + 1x2)
IMROWS = CAP * LI         # 576 rows of this core's images
SROWS = CAP * LW          # 800 rows of this core's captions
COMBO_ROWS = IMROWS + SROWS + 2   # + maskneg row + wdl row
LAM = 9.0
MARGIN = 0.2
MASKNEG = -30000.0

F32 = mybir.dt.float32
F32R = mybir.dt.float32r
F16 = mybir.dt.float16
I8 = mybir.dt.int8

NCH = [(0, 512), (512, WF)]

# Upload im/s as per-row int8 (half the bytes of fp16); dequantize to fp16 on
# device.  Row scales ride along in the f16 aux array.
USE_INT8 = True

_CACHE = {}


def _build_program():
    nc = bass.Bass("TRN2", target_bir_lowering=False, debug=False, num_devices=NC)

    data_dt = I8 if USE_INT8 else F16
    data = nc.dram_tensor("data", [IMROWS + SROWS, D], data_dt, kind="ExternalInput")
    # aux rows (f16): 0 maskneg[0:WF], 1 wdl[0:WF], 2 im row scales[0:IMROWS],
    # 3 s row scales[0:WF]
    aux = nc.dram_tensor("aux", [4, D], F16, kind="ExternalInput")
    scores_out = nc.dram_tensor("scores_out", [128, CAP], F32, kind="ExternalOutput")

    with tile.TileContext(nc) as tc:
        with (
            tc.tile_pool(name="const", bufs=1) as cpool,
            tc.tile_pool(name="nat", bufs=2) as natp,
            tc.tile_pool(name="work", bufs=2) as work,
            tc.tile_pool(name="small", bufs=2) as small,
            tc.tile_pool(name="stage", bufs=1) as stage,
            tc.tile_pool(name="pa", bufs=2, space="PSUM") as pa,
            tc.tile_pool(name="pc", bufs=2, space="PSUM") as pc,
            tc.tile_pool(name="dram", bufs=1, space="DRAM") as dram,
        ):
            # ---- on-device constants -----------------------------------------
            ones32 = cpool.tile([128, 128], F32, tag="ones32")
            nc.vector.memset(ones32[:], 1.0)
            eye32 = cpool.tile([128, 128], F32, tag="eye32")
            nc.gpsimd.affine_select(
                eye32[:], ones32[:], pattern=[[-1, 128]],
                compare_op=mybir.AluOpType.is_equal, fill=0.0,
                base=0, channel_multiplier=1,
            )
            eye16 = cpool.tile([128, 128], F16, tag="eye16")
            nc.scalar.copy(eye16[:], eye32[:])
            ones1t = cpool.tile([1, 128], F16, tag="ones1")      # maskneg row lhsT
            nc.scalar.copy(ones1t[:], ones32[0:1, :])
            onesKt = cpool.tile([1, 128], F32R, tag="onesK")     # wfac bcast lhsT
            nc.scalar.copy(onesKt[:], ones32[0:1, :])
            # onesblk[p, g] = 1 iff p // 36 == g  (band predicate 0 <= p-36g <= 35)
            onesblk = cpool.tile([IMG_GRP * LI, IMG_GRP], F32, tag="ob32")
            nc.gpsimd.memset(onesblk[:], 1.0)
            nc.gpsimd.affine_select(
                onesblk[:], onesblk[:], pattern=[[-LI, IMG_GRP]],
                compare_op=mybir.AluOpType.is_ge, fill=0.0, base=0, channel_multiplier=1,
            )
            nc.gpsimd.affine_select(
                onesblk[:], onesblk[:], pattern=[[LI, IMG_GRP]],
                compare_op=mybir.AluOpType.is_ge, fill=0.0, base=LI - 1, channel_multiplier=-1,
            )
            onesblkt = cpool.tile([IMG_GRP * LI, IMG_GRP], F32R, tag="ob")
            nc.scalar.copy(onesblkt[:], onesblk[:])
            # onesblkT[g, k] = 1 iff k // 36 == g; gmask = onesblkT^T @ onesblkT
            onesblkT = cpool.tile([IMG_GRP, IMG_GRP * LI], F32, tag="obT")
            nc.gpsimd.memset(onesblkT[:], 1.0)
            nc.gpsimd.affine_select(
                onesblkT[:], onesblkT[:], pattern=[[1, IMG_GRP * LI]],
                compare_op=mybir.AluOpType.is_ge, fill=0.0, base=0, channel_multiplier=-LI,
            )
            nc.gpsimd.affine_select(
                onesblkT[:], onesblkT[:], pattern=[[-1, IMG_GRP * LI]],
                compare_op=mybir.AluOpType.is_ge, fill=0.0, base=LI - 1, channel_multiplier=LI,
            )
            gmask_ps = pc.tile([IMG_GRP * LI, IMG_GRP * LI], F32, tag="cs")
            nc.tensor.matmul(gmask_ps[:], onesblkT[:], onesblkT[:], start=True, stop=True)
            gmask32 = cpool.tile([IMG_GRP * LI, IMG_GRP * LI], F32, tag="gmask")
            nc.scalar.copy(gmask32[:], gmask_ps[:])

            masknegt = cpool.tile([1, WF], F16, tag="mn")
            nc.sync.dma_start(masknegt[:], aux[0:1, 0:WF])
            wdl16 = cpool.tile([1, WF], F16, tag="wdl16")
            nc.sync.dma_start(wdl16[:], aux[1:2, 0:WF])
            wdlt = cpool.tile([1, WF], F32, tag="wdl")
            nc.scalar.copy(wdlt[:], wdl16[:])

            if USE_INT8:
                # broadcast the int8 row scales to all 128 partitions via PE
                iscale16 = cpool.tile([1, IMROWS], F16, tag="isc")
                nc.sync.dma_start(iscale16[:], aux[2:3, 0:IMROWS])
                sscale16 = cpool.tile([1, WF], F16, tag="ssc")
                nc.sync.dma_start(sscale16[:], aux[3:4, 0:WF])
                iscale_b = cpool.tile([128, IMROWS], F16, tag="iscb")
                for n0, n1 in [(0, 512), (512, IMROWS)]:
                    isc_ps = pc.tile([128, 512], F32, tag="cs")
                    nc.tensor.matmul(isc_ps[:, 0 : n1 - n0], ones1t[:], iscale16[:, n0:n1], start=True, stop=True)
                    nc.scalar.copy(iscale_b[:, n0:n1], isc_ps[:, 0 : n1 - n0])
                sscale_b = cpool.tile([128, WF], F16, tag="sscb")
                for n0, n1 in NCH:
                    ssc_ps = pc.tile([128, 512], F32, tag="cs")
                    nc.tensor.matmul(ssc_ps[:, 0 : n1 - n0], ones1t[:], sscale16[:, n0:n1], start=True, stop=True)
                    nc.scalar.copy(sscale_b[:, n0:n1], ssc_ps[:, 0 : n1 - n0])

            # ---- transpose this core's im slice, AllGather over NeuronLink ---
            imT_raw = cpool.tile([128, 8, IMROWS], F16, tag="imTraw")
            for t in range((IMROWS + 127) // 128):  # 576 = 4*128 + 64
                p0 = t * 128
                pn = min(128, IMROWS - p0)
                if USE_INT8:
                    nat8 = natp.tile([128, D], I8, tag="nat8")
                    nc.sync.dma_start(nat8[0:pn, :], data[p0 : p0 + pn, :])
                    natt = natp.tile([128, D], F16, tag="nat")
                    nc.scalar.copy(natt[0:pn, :], nat8[0:pn, :])
                else:
                    natt = natp.tile([128, D], F16, tag="nat")
                    nc.sync.dma_start(natt[0:pn, :], data[p0 : p0 + pn, :])
                for c in range(8):
                    tp = pc.tile([128, 128], F16, tag="cs")
                    nc.tensor.transpose(
                        tp[:, 0:pn], natt[0:pn, c * 128 : (c + 1) * 128], eye16[0:pn, 0:pn]
                    )
                    nc.scalar.copy(imT_raw[:, c, p0 : p0 + pn], tp[:, 0:pn])
            if USE_INT8:
                nc.vector.tensor_tensor(
                    imT_raw[:], imT_raw[:],
                    iscale_b[:].unsqueeze(1).broadcast_to([128, 8, IMROWS]),
                    op=mybir.AluOpType.mult,
                )
            imT_loc = imT_raw

            ag_in = dram.tile([128, 8 * IMROWS], F16)
            ag_out = dram.tile([NC, 128, 8 * IMROWS], F16, addr_space="Shared")
            nc.sync.dma_start(ag_in[:], imT_loc[:].rearrange("p c k -> p (c k)"))
            nc.gpsimd.collective_compute(
                "AllGather",
                mybir.AluOpType.bypass,
                replica_groups=[list(range(NC))],
                ins=[ag_in.opt()],
                outs=[ag_out.opt()],
            )

            # imT8[d_lo, c, seg*576 + k] = ag_out[seg, d_lo, c*576 + k]
            imT8 = cpool.tile([128, 8, B * LI], F16, tag="imT8")
            for seg in range(NC):
                for c in range(8):
                    nc.sync.dma_start(
                        imT8[:, c, seg * IMROWS : (seg + 1) * IMROWS],
                        ag_out[seg, :, c * IMROWS : (c + 1) * IMROWS],
                    )

            # ---- transpose this core's caption slice -------------------------
            sT_raw = cpool.tile([128, 8, WF], F16, tag="sTraw")
            for t in range((SROWS + 127) // 128):  # 800 = 6*128 + 32
                p0 = t * 128
                pn = min(128, SROWS - p0)
                if USE_INT8:
                    nat8 = natp.tile([128, D], I8, tag="nat8")
                    nc.sync.dma_start(nat8[0:pn, :], data[IMROWS + p0 : IMROWS + p0 + pn, :])
                    natt = natp.tile([128, D], F16, tag="nat")
                    nc.scalar.copy(natt[0:pn, :], nat8[0:pn, :])
                else:
                    natt = natp.tile([128, D], F16, tag="nat")
                    nc.sync.dma_start(natt[0:pn, :], data[IMROWS + p0 : IMROWS + p0 + pn, :])
                for c in range(8):
                    tp = pc.tile([128, 128], F16, tag="cs")
                    nc.tensor.transpose(
                        tp[:, 0:pn], natt[0:pn, c * 128 : (c + 1) * 128], eye16[0:pn, 0:pn]
                    )
                    nc.scalar.copy(sT_raw[:, c, p0 : p0 + pn], tp[:, 0:pn])
            if USE_INT8:
                nc.vector.tensor_tensor(
                    sT_raw[:], sT_raw[:],
                    sscale_b[:].unsqueeze(1).broadcast_to([128, 8, WF]),
                    op=mybir.AluOpType.mult,
                )
            sT8 = sT_raw

            # ---- caption word norms -> wfac row -> broadcast to 128 rows -----
            # capn2[w] = sum_d s[w,d]^2 via ones^T @ (sT^2), accumulated over
            # the 8 D-chunks.
            capn2_ps = pc.tile([1, WF], F32, tag="cs")
            for c in range(8):
                sq = work.tile([128, WF], F32, tag="e")
                nc.scalar.activation(sq[:], sT8[:, c, :], mybir.ActivationFunctionType.Square)
                for n0, n1 in NCH:
                    nc.tensor.matmul(
                        capn2_ps[:, n0:n1], ones32[:, 0:1], sq[:, n0:n1],
                        start=(c == 0), stop=(c == 7),
                    )
            # wfac = (wmask/len) / sqrt(capn2)  (reuses work-pool slots)
            wf0 = work.tile([1, WF], F32, tag="am")
            nc.scalar.sqrt(wf0[:], capn2_ps[:])
            wf1 = work.tile([1, WF], F32, tag="sub")
            nc.vector.reciprocal(wf1[:], wf0[:])
            wfacv = work.tile([1, WF], F32, tag="e")
            nc.vector.tensor_tensor(wfacv[:], wf1[:], wdlt[:], op=mybir.AluOpType.mult)
            wfacr = work.tile([1, WF], F32R, tag="m")
            nc.scalar.copy(wfacr[:], wfacv[:])
            bc_ps = pc.tile([128, WF], F32, tag="cs")
            for n0, n1 in NCH:
                nc.tensor.matmul(bc_ps[:, n0:n1], onesKt[:], wfacr[:, n0:n1], start=True, stop=True)
            wfacb = cpool.tile([128, WF], F32, tag="wfacb")
            nc.scalar.copy(wfacb[:], bc_ps[:])

            nst = stage.tile([128, WF], F32, tag="nst")
            wst = stage.tile([128, WF], F32, tag="wst")

            # ---- main loop over image groups ---------------------------------
            for b in range(NB):
                ng = min(IMG_GRP, B - b * IMG_GRP)   # images in this group
                P = ng * LI                          # partitions used
                goff = b * IMG_GRP * LI

                # A[P, WF] = im_g @ s^T (+ word-mask row), fp16 PE, f32 PSUM
                a_ps = pa.tile([P, WF], F32, tag="AT")
                for n0, n1 in NCH:
                    for c in range(8):
                        nc.tensor.matmul(
                            a_ps[:, n0:n1], imT8[:, c, goff : goff + P], sT8[:, c, n0:n1],
                            start=(c == 0), stop=False,
                        )
                    nc.tensor.matmul(
                        a_ps[:, n0:n1], ones1t[0:1, 0:P], masknegt[0:1, n0:n1],
                        start=False, stop=True,
                    )

                # block-diagonal Gram of this group's images (zeroed cross terms)
                g_ps = pc.tile([IMG_GRP * LI, IMG_GRP * LI], F32, tag="cs")
                for c in range(8):
                    nc.tensor.matmul(
                        g_ps[0:P, 0:P], imT8[:, c, goff : goff + P], imT8[:, c, goff : goff + P],
                        start=(c == 0), stop=(c == 7),
                    )
                g16 = small.tile([IMG_GRP * LI, IMG_GRP * LI], F16, tag="g16")
                nc.vector.tensor_tensor(g16[0:P, 0:P], g_ps[0:P, 0:P], gmask32[0:P, 0:P], op=mybir.AluOpType.mult)

                am = work.tile([P, WF], F32, tag="am")
                nc.scalar.copy(am[:], a_ps[:])
                # word softmax with per-(row, caption) max subtraction
                mx = small.tile([P, CAP], F32, tag="mx")
                nc.vector.tensor_reduce(
                    mx[:], a_ps[:].rearrange("p (c w) -> p c w", c=CAP, w=LW),
                    axis=mybir.AxisListType.X, op=mybir.AluOpType.max,
                )
                sub = work.tile([P, WF], F32, tag="sub")
                nc.gpsimd.tensor_tensor(
                    sub[:].rearrange("p (c w) -> p c w", c=CAP, w=LW),
                    am[:].rearrange("p (c w) -> p c w", c=CAP, w=LW),
                    mx[:].unsqueeze(2).broadcast_to([P, CAP, LW]),
                    op=mybir.AluOpType.subtract,
                )
                e = work.tile([P, WF], F32, tag="e")
                nc.scalar.activation(e[:], sub[:], mybir.ActivationFunctionType.Exp)
                z = small.tile([P, CAP], F32, tag="z")
                nc.vector.tensor_reduce(
                    z[:], e[:].rearrange("p (c w) -> p c w", c=CAP, w=LW),
                    axis=mybir.AxisListType.X, op=mybir.AluOpType.add,
                )
                rz = small.tile([P, CAP], F32, tag="rz")
                nc.vector.reciprocal(rz[:], z[:])
                m = work.tile([P, WF], F32, tag="m")
                nc.vector.tensor_tensor(
                    m[:].rearrange("p (c w) -> p c w", c=CAP, w=LW),
                    e[:].rearrange("p (c w) -> p c w", c=CAP, w=LW),
                    rz[:].unsqueeze(2).broadcast_to([P, CAP, LW]),
                    op=mybir.AluOpType.mult,
                )
                # E2 = exp(lam * a1): fp16 copy feeds the PE, f32r copy the DVE
                e2h = work.tile([P, WF], F16, tag="e2h")
                nc.scalar.activation(e2h[:], m[:], mybir.ActivationFunctionType.Exp, bias=0.0, scale=LAM)
                e2f = work.tile([P, WF], F32R, tag="e2f")
                nc.scalar.activation(e2f[:], m[:], mybir.ActivationFunctionType.Exp, bias=0.0, scale=LAM)

                f = work.tile([P, WF], F32R, tag="f")
                nc.gpsimd.tensor_tensor(f[:], am[:], e2f[:], op=mybir.AluOpType.mult)

                t_ps = pa.tile([P, WF], F32, tag="AT")
                for n0, n1 in NCH:
                    nc.tensor.matmul(t_ps[:, n0:n1], g16[0:P, 0:P], e2h[:, n0:n1], start=True, stop=True)
                u = work.tile([P, WF], F32R, tag="u")
                nc.vector.tensor_tensor(u[:], t_ps[:], e2f[:], op=mybir.AluOpType.mult)

                n_ps = pc.tile([IMG_GRP, WF], F32, tag="cs")
                for n0, n1 in NCH:
                    nc.tensor.matmul(n_ps[0:ng, n0:n1], onesblkt[0:P, 0:ng], f[:, n0:n1], start=True, stop=True)
                w_ps = pc.tile([IMG_GRP, WF], F32, tag="cs")
                for n0, n1 in NCH:
                    nc.tensor.matmul(w_ps[0:ng, n0:n1], onesblkt[0:P, 0:ng], u[:, n0:n1], start=True, stop=True)

                r0 = b * IMG_GRP
                nb_sb = small.tile([IMG_GRP, WF], F32, tag="nb_sb")
                wb_sb = small.tile([IMG_GRP, WF], F32, tag="wb_sb")
                nc.scalar.copy(nb_sb[0:ng, :], n_ps[0:ng, :])
                nc.scalar.copy(wb_sb[0:ng, :], w_ps[0:ng, :])
                nc.sync.dma_start(nst[r0 : r0 + ng, :], nb_sb[0:ng, :])
                nc.sync.dma_start(wst[r0 : r0 + ng, :], wb_sb[0:ng, :])

            # ---- finalize: scores block [128 images, 16 captions] ------------
            srt = work.tile([128, WF], F32, tag="am")
            nc.scalar.sqrt(srt[:], wst[:])
            q = work.tile([128, WF], F32, tag="e")
            nc.vector.tensor_tensor(q[:], nst[:], wfacb[:], op=mybir.AluOpType.mult)
            rsq = work.tile([128, WF], F32, tag="sub")
            nc.vector.reciprocal(rsq[:], srt[:])
            cosq = work.tile([128, WF], F32, tag="m")
            nc.vector.tensor_tensor(cosq[:], q[:], rsq[:], op=mybir.AluOpType.mult)
            sim = small.tile([128, CAP], F32, tag="sim")
            nc.vector.tensor_reduce(
                sim[:], cosq[:].rearrange("p (c w) -> p c w", c=CAP, w=LW),
                axis=mybir.AxisListType.X, op=mybir.AluOpType.add,
            )
            nc.sync.dma_start(scores_out[:], sim[:])

    return nc


# ---------------------------------------------------------------------------
# Host side
# ---------------------------------------------------------------------------
def _host_prep(im, s, s_l):
    """Build the upload arrays, already concatenated across cores.

    USE_INT8: data [8*(576+800), 1024] int8 (per-row quantized im & s) and
    aux [8*4, 1024] f16 (maskneg row, wmask/len row, im scales, s scales).
    Otherwise a single f16 data array with im & s and the same aux."""
    im = np.asarray(im, dtype=np.float32).reshape(NC, IMROWS, D)
    s = np.asarray(s, dtype=np.float32).reshape(NC, SROWS, D)
    s_l = np.asarray(s_l).astype(np.int64)

    wmask = (np.arange(LW)[None, :] < s_l[:, None])              # [B, LW]
    maskneg_all = ((~wmask) * np.float32(MASKNEG)).astype(np.float16)
    wdl_all = (wmask / s_l[:, None].astype(np.float32)).astype(np.float16)

    aux = np.zeros((NC, 4, D), dtype=np.float16)
    aux[:, 0, 0:WF] = maskneg_all.reshape(NC, WF)
    aux[:, 1, 0:WF] = wdl_all.reshape(NC, WF)

    if not USE_INT8:
        data = _CACHE.get("data_buf")
        if data is None or data.dtype != np.float16:
            data = np.empty((NC, IMROWS + SROWS, D), dtype=np.float16)
            _CACHE["data_buf"] = data
        data[:, 0:IMROWS, :] = im
        data[:, IMROWS:, :] = s
        return {"data": data.reshape(NC * (IMROWS + SROWS), D),
                "aux": aux.reshape(NC * 4, D)}

    data = _CACHE.get("data_buf")
    if data is None or data.dtype != np.int8:
        data = np.empty((NC, IMROWS + SROWS, D), dtype=np.int8)
        _CACHE["data_buf"] = data

    from concurrent.futures import ThreadPoolExecutor
    if "pool" not in _CACHE:
        _CACHE["pool"] = ThreadPoolExecutor(max_workers=NC)
    pool = _CACHE["pool"]

    iscales = np.empty((NC, IMROWS), dtype=np.float16)
    sscales = np.empty((NC, SROWS), dtype=np.float16)

    def quant_core(c):
        x = im[c]
        amax = np.maximum(np.max(np.abs(x), axis=1), 1e-12)
        q = np.rint(x * (np.float32(127.0) / amax)[:, None])
        data[c, 0:IMROWS, :] = q.astype(np.int8)
        iscales[c] = (amax * np.float32(1.0 / 127.0)).astype(np.float16)
        y = s[c]
        amay = np.maximum(np.max(np.abs(y), axis=1), 1e-12)
        qs = np.rint(y * (np.float32(127.0) / amay)[:, None])
        data[c, IMROWS:, :] = qs.astype(np.int8)
        sscales[c] = (amay * np.float32(1.0 / 127.0)).astype(np.float16)

    list(pool.map(quant_core, range(NC)))
    aux[:, 2, 0:IMROWS] = iscales
    aux[:, 3, 0:WF] = sscales
    return {"data": data.reshape(NC * (IMROWS + SROWS), D),
            "aux": aux.reshape(NC * 4, D)}


def _host_loss(scores):
    """Exact hinge margin loss (max violation) on the full score matrix."""
    scores = scores.astype(np.float32)
    diag = np.diagonal(scores)
    cost_s = np.maximum(MARGIN + scores - diag[:, None], 0.0)
    cost_im = np.maximum(MARGIN + scores - diag[None, :], 0.0)
    np.fill_diagonal(cost_s, 0.0)
    np.fill_diagonal(cost_im, 0.0)
    return np.float32(cost_s.max(axis=1).sum() + cost_im.max(axis=0).sum())


def _make_runner(nc):
    """Persistent jitted SPMD executable (same mechanics as
    bass2jax.run_bass_via_pjrt, but built once and reused across calls)."""
    import warnings
    import jax
    from jax.sharding import Mesh, PartitionSpec
    with warnings.catch_warnings():
        warnings.simplefilter("ignore")
        from jax.experimental.shard_map import shard_map
    from concourse.bass2jax import _bass_exec_p, install_neuronx_cc_hook, partition_id_tensor

    install_neuronx_cc_hook()
    partition_name = nc.partition_id_tensor.name if nc.partition_id_tensor else None

    in_names, out_names, out_avals, out_shapes = [], [], [], []
    for alloc in nc.m.functions[0].allocations:
        if not isinstance(alloc, mybir.MemoryLocationSet):
            continue
        name = alloc.memorylocations[0].name
        if alloc.kind == "ExternalInput":
            if name != partition_name:
                in_names.append(name)
        elif alloc.kind == "ExternalOutput":
            shape = tuple(alloc.tensor_shape)
            dtype = mybir.dt.np(alloc.dtype)
            out_names.append(name)
            out_avals.append(jax.core.ShapedArray(shape, dtype))
            out_shapes.append((shape, dtype))
    n_params = len(in_names)
    n_outs = len(out_names)
    in_names_full = in_names + out_names
    if partition_name is not None:
        in_names_full.append(partition_name)
    donate = tuple(range(n_params, n_params + n_outs))

    def _body(*args):
        operands = list(args)
        if partition_name is not None:
            operands.append(partition_id_tensor())
        outs = _bass_exec_p.bind(
            *operands,
            out_avals=tuple(out_avals),
            in_names=tuple(in_names_full),
            out_names=tuple(out_names),
            lowering_input_output_aliases=(),
            sim_require_finite=True,
            sim_require_nnan=True,
            nc=nc,
        )
        return tuple(outs)

    devices = jax.devices()[:NC]
    assert len(devices) == NC
    mesh = Mesh(np.asarray(devices), ("core",))
    in_specs = (PartitionSpec("core"),) * (n_params + n_outs)
    out_specs = (PartitionSpec("core"),) * n_outs
    sharded = jax.jit(
        shard_map(_body, mesh=mesh, in_specs=in_specs, out_specs=out_specs, check_rep=False),
        donate_argnums=donate,
        keep_unused=True,
    )

    def call(global_in_map):
        ins = [np.ascontiguousarray(global_in_map[name]) for name in in_names]
        zeros = [np.zeros((NC * sh[0], *sh[1:]), dt) for sh, dt in out_shapes]
        outs = sharded(*ins, *zeros)
        return {
            name: np.asarray(outs[i]).reshape(NC, *out_shapes[i][0])
            for i, name in enumerate(out_names)
        }

    return call


class _Res:
    """Minimal stand-in for BassKernelResults (test.py reads exec_time_ns)."""
    exec_time_ns = None


def run(im, s, s_l, trace=False):
    """Returns (loss_scalar, scores[128,128], res)."""
    _install_patches()
    if "nc" not in _CACHE:
        _CACHE["nc"] = _build_program()
    nc = _CACHE["nc"]
    in_map = _host_prep(im, s, s_l)

    out = None
    if "runner" not in _CACHE and not _CACHE.get("runner_failed"):
        try:
            _CACHE["runner"] = _make_runner(nc)
        except Exception:
            _CACHE["runner_failed"] = True
    if "runner" in _CACHE:
        try:
            out = _CACHE["runner"](in_map)
        except Exception:
            _CACHE.pop("runner", None)
            _CACHE["runner_failed"] = True
            out = None
    if out is None:
        # Fallback: stock per-call path.
        from concourse.bass_utils import run_bass_kernel_spmd
        per_core = [
            {k: v.reshape(NC, v.shape[0] // NC, *v.shape[1:])[c] for k, v in in_map.items()}
            for c in range(NC)
        ]
        res = run_bass_kernel_spmd(nc, per_core, list(range(NC)), trace=False)
        blocks = np.stack([res.results[c]["scores_out"] for c in range(NC)])
    else:
        blocks = out["scores_out"]                      # [NC, 128, CAP]

    scores = blocks.transpose(1, 0, 2).reshape(128, 128)
    loss = _host_loss(scores)
    return loss, scores, _Res()


def kernel(im, s, s_l):
    loss, _, _ = run(im, s, s_l)
    return np.array(loss, dtype=np.float32)


# revision 24
# speedup vs baseline: 13.0759x; 1.2162x over previous
"""Trainium2 Bass kernel for nn_ContrastiveLoss (stacked cross-attention t2i).

Strategy (8 NeuronCores, caption-sharded; transfer-optimized):
  The axon loopback relay moves host<->device data at only ~38 MB/s, so the
  end-to-end time is dominated by upload bytes.  This version uploads one
  fp16 array per core (~2.8 MB: its 16 images + its 16 captions + two aux
  rows) instead of the ~25 MB fp32 the old kernel shipped:
    - im is uploaded SHARDED (1/8 per core) and AllGathered on-device over
      NeuronLink.
    - s stays sharded (each core only scores its own 16 captions).
    - transposes (im, s -> D-major), per-image Gram matrices, caption-word
      norms, and the wfac broadcast are all computed on device.
    - A- and T-matmuls run in fp16 (PSUM accumulates fp32); the softmax /
      cosine chain stays fp32, identical in structure to the reference.
  Each core returns its (128 x 16) score columns; the host assembles the
  full 128x128 score matrix and computes the (trivial) hinge margin loss.

Math note: with E2 = exp(lam * a1) (unnormalized region attention),
  cos = (sum_r E2*A) / (cap_n * sqrt(E2^T G E2)) exactly, because the region
softmax normalizer cancels between numerator and |weighted context|.
"""

import numpy as np

import concourse.bass as bass
import concourse.tile as tile
from concourse import mybir
from concourse.vector_clock import ScopedClock

# ---------------------------------------------------------------------------
# Workaround for this toolchain: walrus rejects instructions carrying more
# than one semaphore wait.  Split extra waits onto standalone EventSemaphore
# instructions (the same thing wait_ge emits) just before the offender.
# ---------------------------------------------------------------------------
_PATCHED = False


def _install_patches():
    global _PATCHED
    if _PATCHED:
        return
    _PATCHED = True

    def _drain_and_barrier(self, tick_clock, wait_clock):
        nc = self.nc
        drain_inst = nc.sync.drain()
        wait_clock.add_sem_waits(
            drain_inst.ins, ScopedClock({None: tick_clock.global_clock})
        )
        waits = list(drain_inst.ins.sync_info.on_wait)
        if len(waits) > 1:
            drain_inst.ins.sync_info.on_wait = waits[:1]
            for w in waits[1:]:
                extra = nc.sync.drain()
                extra.ins.sync_info = mybir.SyncInfo(on_wait=[w], on_update=[])
        nc.all_engine_barrier()
        popped = nc._tile_sem_poison_stack.pop()
        assert popped is self._sem_poison
        nc.clear_and_free_semaphores(list(self.sems.allocated().values()))
        nc.all_engine_barrier()

    tile.TileContext._drain_and_barrier = _drain_and_barrier

    import concourse.bass_utils as bass_utils
    import concourse.bass2jax as bass2jax
    import orjson

    _orig_compile = bass_utils.compile_bir_kernel

    def _split_waits_in_bir(bir_json: bytes) -> bytes:
        m = orjson.loads(bir_json)
        for fn in m.get("functions", []):
            for blk in fn.get("blocks", []):
                insts = blk.get("instructions", [])
                new_insts = []
                for ins in insts:
                    si = ins.get("sync_info")
                    waits = (si or {}).get("on_wait") or []
                    if len(waits) > 1:
                        for k, w in enumerate(waits[:-1]):
                            new_insts.append(
                                {
                                    "name": f"{ins['name']}_wsplit{k}",
                                    "opcode": "EventSemaphore",
                                    "engine": ins["engine"],
                                    "ins": [],
                                    "outs": [],
                                    "debug": ins.get("debug"),
                                    "sync_info": {"on_update": [], "on_wait": [w]},
                                }
                            )
                        si["on_wait"] = waits[-1:]
                    new_insts.append(ins)
                blk["instructions"] = new_insts
        return orjson.dumps(m)

    def _patched_compile(bir_json, tmpdir, neff_name="file.neff"):
        return _orig_compile(_split_waits_in_bir(bir_json), tmpdir, neff_name)

    bass_utils.compile_bir_kernel = _patched_compile
    bass2jax.compile_bir_kernel = _patched_compile


# ---------------------------------------------------------------------------
# Problem constants (hardcoded per the task contract).
# ---------------------------------------------------------------------------
B = 128           # images == captions
LI = 36           # image regions
LW = 50           # padded caption words
D = 1024          # feature dim
NC = 8            # cores
CAP = B // NC     # captions per core (16)
WF = CAP * LW     # free width of the batched tiles (800)
IMG_GRP = 3       # images per batch
NB = (B + IMG_GRP - 1) // IMG_GRP  # 43 batches (42x3 + 1x2)
IMROWS = CAP * LI         # 576 rows of this core's images
SROWS = CAP * LW          # 800 rows of this core's captions
COMBO_ROWS = IMROWS + SROWS + 2   # + maskneg row + wdl row
LAM = 9.0
MARGIN = 0.2
MASKNEG = -30000.0

F32 = mybir.dt.float32
F32R = mybir.dt.float32r
F16 = mybir.dt.float16
I8 = mybir.dt.int8

NCH = [(0, 512), (512, WF)]

# Upload im/s as per-row int8 (half the bytes of fp16); dequantize to fp16 on
# device.  Row scales ride along in the f16 aux array.
USE_INT8 = True

_CACHE = {}


def _build_program():
    nc = bass.Bass("TRN2", target_bir_lowering=False, debug=False, num_devices=NC)

    data_dt = I8 if USE_INT8 else F16
    data = nc.dram_tensor("data", [IMROWS + SROWS, D], data_dt, kind="ExternalInput")
    # aux rows (f16): 0 maskneg[0:WF], 1 wdl[0:WF], 2 im row scales[0:IMROWS],
    # 3 s row scales[0:WF]
    aux = nc.dram_tensor("aux", [4, D], F16, kind="ExternalInput")
    scores_out = nc.dram_tensor("scores_out", [128, CAP], F32, kind="ExternalOutput")

    with tile.TileContext(nc) as tc:
        with (
            tc.tile_pool(name="const", bufs=1) as cpool,
            tc.tile_pool(name="nat", bufs=2) as natp,
            tc.tile_pool(name="work", bufs=2) as work,
            tc.tile_pool(name="small", bufs=2) as small,
            tc.tile_pool(name="stage", bufs=1) as stage,
            tc.tile_pool(name="pa", bufs=2, space="PSUM") as pa,
            tc.tile_pool(name="pc", bufs=2, space="PSUM") as pc,
            tc.tile_pool(name="dram", bufs=1, space="DRAM") as dram,
        ):
            # ---- on-device constants -----------------------------------------
            ones32 = cpool.tile([128, 128], F32, tag="ones32")
            nc.vector.memset(ones32[:], 1.0)
            eye32 = cpool.tile([128, 128], F32, tag="eye32")
            nc.gpsimd.affine_select(
                eye32[:], ones32[:], pattern=[[-1, 128]],
                compare_op=mybir.AluOpType.is_equal, fill=0.0,
                base=0, channel_multiplier=1,
            )
            eye16 = cpool.tile([128, 128], F16, tag="eye16")
            nc.scalar.copy(eye16[:], eye32[:])
            ones1t = cpool.tile([1, 128], F16, tag="ones1")      # maskneg row lhsT
            nc.scalar.copy(ones1t[:], ones32[0:1, :])
            onesKt = cpool.tile([1, 128], F32R, tag="onesK")     # wfac bcast lhsT
            nc.scalar.copy(onesKt[:], ones32[0:1, :])
            # onesblk[p, g] = 1 iff p // 36 == g  (band predicate 0 <= p-36g <= 35)
            onesblk = cpool.tile([IMG_GRP * LI, IMG_GRP], F32, tag="ob32")
            nc.gpsimd.memset(onesblk[:], 1.0)
            nc.gpsimd.affine_select(
                onesblk[:], onesblk[:], pattern=[[-LI, IMG_GRP]],
                compare_op=mybir.AluOpType.is_ge, fill=0.0, base=0, channel_multiplier=1,
            )
            nc.gpsimd.affine_select(
                onesblk[:], onesblk[:], pattern=[[LI, IMG_GRP]],
                compare_op=mybir.AluOpType.is_ge, fill=0.0, base=LI - 1, channel_multiplier=-1,
            )
            onesblkt = cpool.tile([IMG_GRP * LI, IMG_GRP], F32R, tag="ob")
            nc.scalar.copy(onesblkt[:], onesblk[:])
            # onesblkT[g, k] = 1 iff k // 36 == g; gmask = onesblkT^T @ onesblkT
            onesblkT = cpool.tile([IMG_GRP, IMG_GRP * LI], F32, tag="obT")
            nc.gpsimd.memset(onesblkT[:], 1.0)
            nc.gpsimd.affine_select(
                onesblkT[:], onesblkT[:], pattern=[[1, IMG_GRP * LI]],
                compare_op=mybir.AluOpType.is_ge, fill=0.0, base=0, channel_multiplier=-LI,
            )
            nc.gpsimd.affine_select(
                onesblkT[:], onesblkT[:], pattern=[[-1, IMG_GRP * LI]],
                compare_op=mybir.AluOpType.is_ge, fill=0.0, base=LI - 1, channel_multiplier=LI,
            )
            gmask_ps = pc.tile([IMG_GRP * LI, IMG_GRP * LI], F32, tag="cs")
            nc.tensor.matmul(gmask_ps[:], onesblkT[:], onesblkT[:], start=True, stop=True)
            gmask32 = cpool.tile([IMG_GRP * LI, IMG_GRP * LI], F32, tag="gmask")
            nc.scalar.copy(gmask32[:], gmask_ps[:])

            masknegt = cpool.tile([1, WF], F16, tag="mn")
            nc.sync.dma_start(masknegt[:], aux[0:1, 0:WF])
            wdl16 = cpool.tile([1, WF], F16, tag="wdl16")
            nc.sync.dma_start(wdl16[:], aux[1:2, 0:WF])
            wdlt = cpool.tile([1, WF], F32, tag="wdl")
            nc.scalar.copy(wdlt[:], wdl16[:])

            if USE_INT8:
                # broadcast the int8 row scales to all 128 partitions via PE
                iscale16 = cpool.tile([1, IMROWS], F16, tag="isc")
                nc.sync.dma_start(iscale16[:], aux[2:3, 0:IMROWS])
                sscale16 = cpool.tile([1, WF], F16, tag="ssc")
                nc.sync.dma_start(sscale16[:], aux[3:4, 0:WF])
                iscale_b = cpool.tile([128, IMROWS], F16, tag="iscb")
                for n0, n1 in [(0, 512), (512, IMROWS)]:
                    isc_ps = pc.tile([128, 512], F32, tag="cs")
                    nc.tensor.matmul(isc_ps[:, 0 : n1 - n0], ones1t[:], iscale16[:, n0:n1], start=True, stop=True)
                    nc.scalar.copy(iscale_b[:, n0:n1], isc_ps[:, 0 : n1 - n0])
                sscale_b = cpool.tile([128, WF], F16, tag="sscb")
                for n0, n1 in NCH:
                    ssc_ps = pc.tile([128, 512], F32, tag="cs")
                    nc.tensor.matmul(ssc_ps[:, 0 : n1 - n0], ones1t[:], sscale16[:, n0:n1], start=True, stop=True)
                    nc.scalar.copy(sscale_b[:, n0:n1], ssc_ps[:, 0 : n1 - n0])

            # ---- transpose this core's im slice, AllGather over NeuronLink ---
            imT_raw = cpool.tile([128, 8, IMROWS], F16, tag="imTraw")
            for t in range((IMROWS + 127) // 128):  # 576 = 4*128 + 64
                p0 = t * 128
                pn = min(128, IMROWS - p0)
                if USE_INT8:
                    nat8 = natp.tile([128, D], I8, tag="nat8")
                    nc.sync.dma_start(nat8[0:pn, :], data[p0 : p0 + pn, :])
                    natt = natp.tile([128, D], F16, tag="nat")
                    nc.scalar.copy(natt[0:pn, :], nat8[0:pn, :])
                else:
                    natt = natp.tile([128, D], F16, tag="nat")
                    nc.sync.dma_start(natt[0:pn, :], data[p0 : p0 + pn, :])
                for c in range(8):
                    tp = pc.tile([128, 128], F16, tag="cs")
                    nc.tensor.transpose(
                        tp[:, 0:pn], natt[0:pn, c * 128 : (c + 1) * 128], eye16[0:pn, 0:pn]
                    )
                    nc.scalar.copy(imT_raw[:, c, p0 : p0 + pn], tp[:, 0:pn])
            if USE_INT8:
                nc.vector.tensor_tensor(
                    imT_raw[:], imT_raw[:],
                    iscale_b[:].unsqueeze(1).broadcast_to([128, 8, IMROWS]),
                    op=mybir.AluOpType.mult,
                )
            imT_loc = imT_raw

            ag_in = dram.tile([128, 8 * IMROWS], F16)
            ag_out = dram.tile([NC, 128, 8 * IMROWS], F16, addr_space="Shared")
            nc.sync.dma_start(ag_in[:], imT_loc[:].rearrange("p c k -> p (c k)"))
            nc.gpsimd.collective_compute(
                "AllGather",
                mybir.AluOpType.bypass,
                replica_groups=[list(range(NC))],
                ins=[ag_in.opt()],
                outs=[ag_out.opt()],
            )

            # imT8[d_lo, c, seg*576 + k] = ag_out[seg, d_lo, c*576 + k]
            imT8 = cpool.tile([128, 8, B * LI], F16, tag="imT8")
            for seg in range(NC):
                for c in range(8):
                    nc.sync.dma_start(
                        imT8[:, c, seg * IMROWS : (seg + 1) * IMROWS],
                        ag_out[seg, :, c * IMROWS : (c + 1) * IMROWS],
                    )

            # ---- transpose this core's caption slice -------------------------
            sT_raw = cpool.tile([128, 8, WF], F16, tag="sTraw")
            for t in range((SROWS + 127) // 128):  # 800 = 6*128 + 32
                p0 = t * 128
                pn = min(128, SROWS - p0)
                if USE_INT8:
                    nat8 = natp.tile([128, D], I8, tag="nat8")
                    nc.sync.dma_start(nat8[0:pn, :], data[IMROWS + p0 : IMROWS + p0 + pn, :])
                    natt = natp.tile([128, D], F16, tag="nat")
                    nc.scalar.copy(natt[0:pn, :], nat8[0:pn, :])
                else:
                    natt = natp.tile([128, D], F16, tag="nat")
                    nc.sync.dma_start(natt[0:pn, :], data[IMROWS + p0 : IMROWS + p0 + pn, :])
                for c in range(8):
                    tp = pc.tile([128, 128], F16, tag="cs")
                    nc.tensor.transpose(
                        tp[:, 0:pn], natt[0:pn, c * 128 : (c + 1) * 128], eye16[0:pn, 0:pn]
                    )
                    nc.scalar.copy(sT_raw[:, c, p0 : p0 + pn], tp[:, 0:pn])
            if USE_INT8:
                nc.vector.tensor_tensor(
                    sT_raw[:], sT_raw[:],
                    sscale_b[:].unsqueeze(1).broadcast_to([128, 8, WF]),
                    op=mybir.AluOpType.mult,
                )
            sT8 = sT_raw

            # ---- caption word norms -> wfac row -> broadcast to 128 rows -----
            # capn2[w] = sum_d s[w,d]^2 via ones^T @ (sT^2), accumulated over
            # the 8 D-chunks.
            capn2_ps = pc.tile([1, WF], F32, tag="cs")
            for c in range(8):
                sq = work.tile([128, WF], F32, tag="e")
                nc.scalar.activation(sq[:], sT8[:, c, :], mybir.ActivationFunctionType.Square)
                for n0, n1 in NCH:
                    nc.tensor.matmul(
                        capn2_ps[:, n0:n1], ones32[:, 0:1], sq[:, n0:n1],
                        start=(c == 0), stop=(c == 7),
                    )
            # wfac = (wmask/len) / sqrt(capn2)  (reuses work-pool slots)
            wf0 = work.tile([1, WF], F32, tag="am")
            nc.scalar.sqrt(wf0[:], capn2_ps[:])
            wf1 = work.tile([1, WF], F32, tag="sub")
            nc.vector.reciprocal(wf1[:], wf0[:])
            wfacv = work.tile([1, WF], F32, tag="e")
            nc.vector.tensor_tensor(wfacv[:], wf1[:], wdlt[:], op=mybir.AluOpType.mult)
            wfacr = work.tile([1, WF], F32R, tag="m")
            nc.scalar.copy(wfacr[:], wfacv[:])
            bc_ps = pc.tile([128, WF], F32, tag="cs")
            for n0, n1 in NCH:
                nc.tensor.matmul(bc_ps[:, n0:n1], onesKt[:], wfacr[:, n0:n1], start=True, stop=True)
            wfacb = cpool.tile([128, WF], F32, tag="wfacb")
            nc.scalar.copy(wfacb[:], bc_ps[:])

            nst = stage.tile([128, WF], F32, tag="nst")
            wst = stage.tile([128, WF], F32, tag="wst")

            # ---- main loop over image groups ---------------------------------
            for b in range(NB):
                ng = min(IMG_GRP, B - b * IMG_GRP)   # images in this group
                P = ng * LI                          # partitions used
                goff = b * IMG_GRP * LI

                # A[P, WF] = im_g @ s^T (+ word-mask row), fp16 PE, f32 PSUM
                a_ps = pa.tile([P, WF], F32, tag="AT")
                for n0, n1 in NCH:
                    for c in range(8):
                        nc.tensor.matmul(
                            a_ps[:, n0:n1], imT8[:, c, goff : goff + P], sT8[:, c, n0:n1],
                            start=(c == 0), stop=False,
                        )
                    nc.tensor.matmul(
                        a_ps[:, n0:n1], ones1t[0:1, 0:P], masknegt[0:1, n0:n1],
                        start=False, stop=True,
                    )

                # block-diagonal Gram of this group's images (zeroed cross terms)
                g_ps = pc.tile([IMG_GRP * LI, IMG_GRP * LI], F32, tag="cs")
                for c in range(8):
                    nc.tensor.matmul(
                        g_ps[0:P, 0:P], imT8[:, c, goff : goff + P], imT8[:, c, goff : goff + P],
                        start=(c == 0), stop=(c == 7),
                    )
                g16 = small.tile([IMG_GRP * LI, IMG_GRP * LI], F16, tag="g16")
                nc.vector.tensor_tensor(g16[0:P, 0:P], g_ps[0:P, 0:P], gmask32[0:P, 0:P], op=mybir.AluOpType.mult)

                am = work.tile([P, WF], F32, tag="am")
                nc.scalar.copy(am[:], a_ps[:])
                # word softmax with per-(row, caption) max subtraction
                mx = small.tile([P, CAP], F32, tag="mx")
                nc.vector.tensor_reduce(
                    mx[:], a_ps[:].rearrange("p (c w) -> p c w", c=CAP, w=LW),
                    axis=mybir.AxisListType.X, op=mybir.AluOpType.max,
                )
                sub = work.tile([P, WF], F32, tag="sub")
                nc.gpsimd.tensor_tensor(
                    sub[:].rearrange("p (c w) -> p c w", c=CAP, w=LW),
                    am[:].rearrange("p (c w) -> p c w", c=CAP, w=LW),
                    mx[:].unsqueeze(2).broadcast_to([P, CAP, LW]),
                    op=mybir.AluOpType.subtract,
                )
                e = work.tile([P, WF], F32, tag="e")
                nc.scalar.activation(e[:], sub[:], mybir.ActivationFunctionType.Exp)
                z = small.tile([P, CAP], F32, tag="z")
                nc.vector.tensor_reduce(
                    z[:], e[:].rearrange("p (c w) -> p c w", c=CAP, w=LW),
                    axis=mybir.AxisListType.X, op=mybir.AluOpType.add,
                )
                rz = small.tile([P, CAP], F32, tag="rz")
                nc.vector.reciprocal(rz[:], z[:])
                m = work.tile([P, WF], F32, tag="m")
                nc.vector.tensor_tensor(
                    m[:].rearrange("p (c w) -> p c w", c=CAP, w=LW),
                    e[:].rearrange("p (c w) -> p c w", c=CAP, w=LW),
                    rz[:].unsqueeze(2).broadcast_to([P, CAP, LW]),
                    op=mybir.AluOpType.mult,
                )
                # E2 = exp(lam * a1): fp16 copy feeds the PE, f32r copy the DVE
                e2h = work.tile([P, WF], F16, tag="e2h")
                nc.scalar.activation(e2h[:], m[:], mybir.ActivationFunctionType.Exp, bias=0.0, scale=LAM)
                e2f = work.tile([P, WF], F32R, tag="e2f")
                nc.scalar.activation(e2f[:], m[:], mybir.ActivationFunctionType.Exp, bias=0.0, scale=LAM)

                f = work.tile([P, WF], F32R, tag="f")
                nc.gpsimd.tensor_tensor(f[:], am[:], e2f[:], op=mybir.AluOpType.mult)

                t_ps = pa.tile([P, WF], F32, tag="AT")
                for n0, n1 in NCH:
                    nc.tensor.matmul(t_ps[:, n0:n1], g16[0:P, 0:P], e2h[:, n0:n1], start=True, stop=True)
                u = work.tile([P, WF], F32R, tag="u")
                nc.vector.tensor_tensor(u[:], t_ps[:], e2f[:], op=mybir.AluOpType.mult)

                n_ps = pc.tile([IMG_GRP, WF], F32, tag="cs")
                for n0, n1 in NCH:
                    nc.tensor.matmul(n_ps[0:ng, n0:n1], onesblkt[0:P, 0:ng], f[:, n0:n1], start=True, stop=True)
                w_ps = pc.tile([IMG_GRP, WF], F32, tag="cs")
                for n0, n1 in NCH:
                    nc.tensor.matmul(w_ps[0:ng, n0:n1], onesblkt[0:P, 0:ng], u[:, n0:n1], start=True, stop=True)

                r0 = b * IMG_GRP
                nb_sb = small.tile([IMG_GRP, WF], F32, tag="nb_sb")
                wb_sb = small.tile([IMG_GRP, WF], F32, tag="wb_sb")
                nc.scalar.copy(nb_sb[0:ng, :], n_ps[0:ng, :])
                nc.scalar.copy(wb_sb[0:ng, :], w_ps[0:ng, :])
                nc.sync.dma_start(nst[r0 : r0 + ng, :], nb_sb[0:ng, :])
                nc.sync.dma_start(wst[r0 : r0 + ng, :], wb_sb[0:ng, :])

            # ---- finalize: scores block [128 images, 16 captions] ------------
            srt = work.tile([128, WF], F32, tag="am")
            nc.scalar.sqrt(srt[:], wst[:])
            q = work.tile([128, WF], F32, tag="e")
            nc.vector.tensor_tensor(q[:], nst[:], wfacb[:], op=mybir.AluOpType.mult)
            rsq = work.tile([128, WF], F32, tag="sub")
            nc.vector.reciprocal(rsq[:], srt[:])
            cosq = work.tile([128, WF], F32, tag="m")
            nc.vector.tensor_tensor(cosq[:], q[:], rsq[:], op=mybir.AluOpType.mult)
            sim = small.tile([128, CAP], F32, tag="sim")
            nc.vector.tensor_reduce(
                sim[:], cosq[:].rearrange("p (c w) -> p c w", c=CAP, w=LW),
                axis=mybir.AxisListType.X, op=mybir.AluOpType.add,
            )
            nc.sync.dma_start(scores_out[:], sim[:])

    return nc


# ---------------------------------------------------------------------------
# Host side
# ---------------------------------------------------------------------------
def _host_prep(im, s, s_l, uploader=None):
    """Build the upload arrays, already concatenated across cores.

    USE_INT8: data [8*(576+800), 1024] int8 (per-row quantized im & s) and
    aux [8*4, 1024] f16 (maskneg row, wmask/len row, im scales, s scales).
    Otherwise a single f16 data array with im & s and the same aux.

    If `uploader` (per-core-shard async device_put from the runner) is given,
    each core's data shard is shipped as soon as its quantization finishes so
    CPU prep overlaps the serialized host->device stream; "data" is then a
    committed jax Array."""
    im = np.asarray(im, dtype=np.float32).reshape(NC, IMROWS, D)
    s = np.asarray(s, dtype=np.float32).reshape(NC, SROWS, D)
    s_l = np.asarray(s_l).astype(np.int64)

    wmask = (np.arange(LW)[None, :] < s_l[:, None])              # [B, LW]
    maskneg_all = ((~wmask) * np.float32(MASKNEG)).astype(np.float16)
    wdl_all = (wmask / s_l[:, None].astype(np.float32)).astype(np.float16)

    aux = np.zeros((NC, 4, D), dtype=np.float16)
    aux[:, 0, 0:WF] = maskneg_all.reshape(NC, WF)
    aux[:, 1, 0:WF] = wdl_all.reshape(NC, WF)

    want_dt = np.int8 if USE_INT8 else np.float16
    data = _CACHE.get("data_buf")
    if data is None or data.dtype != want_dt:
        data = np.empty((NC, IMROWS + SROWS, D), dtype=want_dt)
        _CACHE["data_buf"] = data

    iscales = np.empty((NC, IMROWS), dtype=np.float16)
    sscales = np.empty((NC, SROWS), dtype=np.float16)
    shards = [None] * NC

    def quant_core(c):
        if USE_INT8:
            x = im[c]
            amax = np.maximum(np.max(np.abs(x), axis=1), 1e-12)
            data[c, 0:IMROWS, :] = x * (np.float32(127.0) / amax)[:, None]
            iscales[c] = (amax * np.float32(1.0 / 127.0)).astype(np.float16)
            y = s[c]
            amay = np.maximum(np.max(np.abs(y), axis=1), 1e-12)
            data[c, IMROWS:, :] = y * (np.float32(127.0) / amay)[:, None]
            sscales[c] = (amay * np.float32(1.0 / 127.0)).astype(np.float16)
        else:
            data[c, 0:IMROWS, :] = im[c]
            data[c, IMROWS:, :] = s[c]
        if uploader is not None:
            shards[c] = uploader(c, data[c])

    from concurrent.futures import ThreadPoolExecutor
    if "pool" not in _CACHE:
        _CACHE["pool"] = ThreadPoolExecutor(max_workers=NC)
    list(_CACHE["pool"].map(quant_core, range(NC)))

    if USE_INT8:
        aux[:, 2, 0:IMROWS] = iscales
        aux[:, 3, 0:WF] = sscales
    data_out = data.reshape(NC * (IMROWS + SROWS), D)
    return {"data": data_out, "aux": aux.reshape(NC * 4, D)}, shards


def _host_loss(scores):
    """Exact hinge margin loss (max violation) on the full score matrix."""
    scores = scores.astype(np.float32)
    diag = np.diagonal(scores)
    cost_s = np.maximum(MARGIN + scores - diag[:, None], 0.0)
    cost_im = np.maximum(MARGIN + scores - diag[None, :], 0.0)
    np.fill_diagonal(cost_s, 0.0)
    np.fill_diagonal(cost_im, 0.0)
    return np.float32(cost_s.max(axis=1).sum() + cost_im.max(axis=0).sum())


def _make_runner(nc):
    """Persistent jitted SPMD executable (same mechanics as
    bass2jax.run_bass_via_pjrt, but built once and reused across calls)."""
    import warnings
    import jax
    from jax.sharding import Mesh, PartitionSpec
    with warnings.catch_warnings():
        warnings.simplefilter("ignore")
        from jax.experimental.shard_map import shard_map
    from concourse.bass2jax import _bass_exec_p, install_neuronx_cc_hook, partition_id_tensor

    install_neuronx_cc_hook()
    partition_name = nc.partition_id_tensor.name if nc.partition_id_tensor else None

    in_names, out_names, out_avals, out_shapes = [], [], [], []
    for alloc in nc.m.functions[0].allocations:
        if not isinstance(alloc, mybir.MemoryLocationSet):
            continue
        name = alloc.memorylocations[0].name
        if alloc.kind == "ExternalInput":
            if name != partition_name:
                in_names.append(name)
        elif alloc.kind == "ExternalOutput":
            shape = tuple(alloc.tensor_shape)
            dtype = mybir.dt.np(alloc.dtype)
            out_names.append(name)
            out_avals.append(jax.core.ShapedArray(shape, dtype))
            out_shapes.append((shape, dtype))
    n_params = len(in_names)
    n_outs = len(out_names)
    in_names_full = in_names + out_names
    if partition_name is not None:
        in_names_full.append(partition_name)
    donate = tuple(range(n_params, n_params + n_outs))

    def _body(*args):
        operands = list(args)
        if partition_name is not None:
            operands.append(partition_id_tensor())
        outs = _bass_exec_p.bind(
            *operands,
            out_avals=tuple(out_avals),
            in_names=tuple(in_names_full),
            out_names=tuple(out_names),
            lowering_input_output_aliases=(),
            sim_require_finite=True,
            sim_require_nnan=True,
            nc=nc,
        )
        return tuple(outs)

    devices = jax.devices()[:NC]
    assert len(devices) == NC
    mesh = Mesh(np.asarray(devices), ("core",))
    in_specs = (PartitionSpec("core"),) * (n_params + n_outs)
    out_specs = (PartitionSpec("core"),) * n_outs
    sharded = jax.jit(
        shard_map(_body, mesh=mesh, in_specs=in_specs, out_specs=out_specs, check_rep=False),
        donate_argnums=donate,
        keep_unused=True,
    )
    data_sharding = jax.sharding.NamedSharding(mesh, PartitionSpec("core"))

    def uploader(c, shard_np):
        # async single-shard transfer; assembled by assemble() below
        return jax.device_put(shard_np, devices[c])

    def assemble(shards, per_shard_shape):
        global_shape = (NC * per_shard_shape[0], *per_shard_shape[1:])
        return jax.make_array_from_single_device_arrays(global_shape, data_sharding, shards)

    def call(global_in_map):
        ins = []
        for name in in_names:
            v = global_in_map[name]
            ins.append(v if isinstance(v, jax.Array) else np.ascontiguousarray(v))
        zeros = [np.zeros((NC * sh[0], *sh[1:]), dt) for sh, dt in out_shapes]
        outs = sharded(*ins, *zeros)
        return {
            name: np.asarray(outs[i]).reshape(NC, *out_shapes[i][0])
            for i, name in enumerate(out_names)
        }

    call.uploader = uploader
    call.assemble = assemble
    return call


class _Res:
    """Minimal stand-in for BassKernelResults (test.py reads exec_time_ns)."""
    exec_time_ns = None


def run(im, s, s_l, trace=False):
    """Returns (loss_scalar, scores[128,128], res)."""
    _install_patches()
    if "nc" not in _CACHE:
        _CACHE["nc"] = _build_program()
    nc = _CACHE["nc"]

    if "runner" not in _CACHE and not _CACHE.get("runner_failed"):
        try:
            _CACHE["runner"] = _make_runner(nc)
        except Exception:
            _CACHE["runner_failed"] = True
    runner = _CACHE.get("runner")

    out = None
    if runner is not None:
        try:
            in_map, shards = _host_prep(im, s, s_l, uploader=runner.uploader)
            if all(sh is not None for sh in shards):
                in_map["data"] = runner.assemble(shards, (IMROWS + SROWS, D))
            out = runner(in_map)
        except Exception:
            _CACHE.pop("runner", None)
            _CACHE["runner_failed"] = True
            out = None
    if out is None:
        # Fallback: stock per-call path.
        from concourse.bass_utils import run_bass_kernel_spmd
        in_map, _ = _host_prep(im, s, s_l)
        per_core = [
            {k: v.reshape(NC, v.shape[0] // NC, *v.shape[1:])[c] for k, v in in_map.items()}
            for c in range(NC)
        ]
        res = run_bass_kernel_spmd(nc, per_core, list(range(NC)), trace=False)
        blocks = np.stack([res.results[c]["scores_out"] for c in range(NC)])
    else:
        blocks = out["scores_out"]                      # [NC, 128, CAP]

    scores = blocks.transpose(1, 0, 2).reshape(128, 128)
    loss = _host_loss(scores)
    return loss, scores, _Res()


def kernel(im, s, s_l):
    loss, _, _ = run(im, s, s_l)
    return np.array(loss, dtype=np.float32)
